# revision 75
# baseline (speedup 1.0000x reference)
"""Distributed Trainium2 Bass kernel for nn_CrossAttention (B=4, L=1024,
Lc=2048, C=1024, H=16).

Sharding: 8 cores = 4 batches x 2 head-groups of 8 heads. Each core
computes its batch's q/k/v projections for its 8 heads, the attention,
and a partial output projection (row-shard of Wp). Host sums the two
partial outputs per batch (bf16) and adds bp.

Precision: q/k projections and the S=q@k^T matmul run in fp8e4 with
perf_mode=DoubleRow (two contraction subtiles per instruction -> 2x PE
throughput, K=256 per instruction for the projections). Wq/Wk are
pre-scaled by 32 on the host (q/k are l2-normalized downstream so the
scale cancels); qhat is rescaled by R=2048 into fp8 range and exp()
compensates with scale=1/R. Weight columns are reordered host-side so
each 128-partition tile holds 4 heads x 32 head-dim lanes, giving the
S matmul its [32, 2, *] DoubleRow layout directly. V/AV/P-projection
stay bf16 for accuracy.

Schedule: per head, S -> exp -> bias-multiply m-tiles stream at the
scalar engine's pace (exp is the global bottleneck at ~134us busy).
AV matmuls trail roughly one head behind via a global cursor so exp
never stalls on PE; the v projection interleaves into heads 0-1; the
first half of the output projection (head pairs j=0,1) runs in-stream
into an SBUF partial (ypart) once heads 0-3 finish; heads run in order
[0,1,2,3,4,5,7,6] so the final tail avoids the cross-partition DMA.
The softmax denominator rides as a leading ones-column in V (rowsum
lands at PSUM partition 0 for partition_broadcast); values sit at
partitions 32..95 to satisfy base-partition rules.
"""

import os
import sys
from contextlib import ExitStack

sys.path.insert(0, "/opt/trn_rl_repo")

import numpy as np
import ml_dtypes

import concourse.bass as bass
from concourse import bacc
import concourse.mybir as mybir
import concourse.tile as tile
from concourse.bass_utils import run_bass_kernel_spmd

BF16 = ml_dtypes.bfloat16
F8 = ml_dtypes.float8_e4m3
AF = mybir.ActivationFunctionType
ALU = mybir.AluOpType
AX = mybir.AxisListType
PM = mybir.MatmulPerfMode

# All ACT functions used here (Copy/Exp/Ln) live in the
# natural_log_exp_and_others table set; blank the other sets so
# insert_act_table_loads emits exactly one table load.
from concourse.hw_specs import get_activation_tables as _gat_orig


def _gat_one_set(arch):
    t = _gat_orig(arch)
    return {
        k: (v if k == "natural_log_exp_and_others" else set()) for k, v in t.items()
    }


bacc.get_activation_tables = _gat_one_set

B, L, LC, C, H = 4, 1024, 2048, 1024, 16
HG = 8  # heads per core
D = 64  # head dim
OC = HG * D  # 512 output channels per core
N_CORES = 8
MAX_SCALE_MUL = float(np.log(100.0))
WSCALE = 32.0  # host pre-scale on Wq/Wk before fp8 quantization
R = 2048.0  # qhat rescale into fp8 range; exp() applies 1/R

# module-level knobs for test harness
TRACE = False
LAST_RESULT = None

_NC_CACHE = {}


def build_nc():
    f32, bf16, f8 = mybir.dt.float32, mybir.dt.bfloat16, mybir.dt.float8e4
    nc = bacc.Bacc()

    xT8 = nc.declare_dram_parameter("xT8", [C, L], f8, isOutput=False)
    ctxT = nc.declare_dram_parameter("ctxT", [C, LC], bf16, isOutput=False)
    ctxT8 = nc.declare_dram_parameter("ctxT8", [C, LC], f8, isOutput=False)
    wqT = nc.declare_dram_parameter("wqT", [C, OC], f8, isOutput=False)
    wkT = nc.declare_dram_parameter("wkT", [C, OC], f8, isOutput=False)
    wvT = nc.declare_dram_parameter("wvT", [C, OC], bf16, isOutput=False)
    wpT = nc.declare_dram_parameter("wpT", [OC, C], bf16, isOutput=False)
    expbT = nc.declare_dram_parameter("expbT", [HG, LC, L], bf16, isOutput=False)
    hsum = nc.declare_dram_parameter("hsum", [OC, HG], bf16, isOutput=False)
    hbc = nc.declare_dram_parameter("hbc", [HG, OC], bf16, isOutput=False)
    sminv = nc.declare_dram_parameter("sminv", [HG, 1], f32, isOutput=False)
    y = nc.declare_dram_parameter("y", [L, C], bf16, isOutput=True)
    dbg = {}
    if os.environ.get("KDBG", "0") == "1":
        dbg["qhatF"] = nc.declare_dram_parameter("d_qhatF", [128, 2, L], f8, isOutput=True)
        dbg["kTF"] = nc.declare_dram_parameter("d_kTF", [128, 2, LC], f8, isOutput=True)
        dbg["ssb"] = nc.declare_dram_parameter("d_ssb", [HG, L], bf16, isOutput=True)
        dbg["rsk"] = nc.declare_dram_parameter("d_rsk", [128, 4], f32, isOutput=True)
        dbg["qT0"] = nc.declare_dram_parameter("d_qT0", [128, L], bf16, isOutput=True)
        dbg["v0"] = nc.declare_dram_parameter("d_v0", [128, HG, 96], bf16, isOutput=True)
        dbg["praw"] = nc.declare_dram_parameter("d_praw", [128, L], bf16, isOutput=True)
        dbg["ptb"] = nc.declare_dram_parameter("d_ptb", [128, L], bf16, isOutput=True)
        dbg["osb"] = nc.declare_dram_parameter("d_osb", [96, L], f32, isOutput=True)
        dbg["on2"] = nc.declare_dram_parameter("d_on2", [128, L], bf16, isOutput=True)

    KT = C // 128  # 8 contraction tiles (DoubleRow: pairs -> 4 steps)
    OCT = OC // 128  # 4 output-channel tiles
    MT = LC // 128  # 16 context tiles
    LT = L // 128  # 8 query tiles

    with tile.TileContext(nc) as tc, ExitStack() as persist:
        keep = persist.enter_context(tc.tile_pool(name="keep", bufs=1))
        dma = nc.sync

        # head PAIRS stacked across the 128 partitions for the output
        # projection: contraction becomes standard K=128 matmuls
        wp_t = keep.tile([128, HG // 2, C], bf16, tag="wp")
        wp2_sb = [wp_t[:, j, :] for j in range(HG // 2)]

        # fp8 DoubleRow layouts for the S matmul:
        # group g in {0,1} holds heads 4g..4g+3; head h4 = partitions
        # 32*h4..32*h4+31, subtile i covers head-dim 32i..32i+31.
        kTF_sb = [keep.tile([128, 2, LC], f8, tag=f"kTF{g}", name=f"kTF{g}") for g in range(2)]
        qhatF_sb = [keep.tile([128, 2, L], f8, tag=f"qhatF{g}", name=f"qhatF{g}") for g in range(2)]
        # matmul operands must have base partition in {0,32,64}; head 3 of
        # each group lives at base 96, so keep partition-shifted copies
        # (group g at base 32*g of one shared tile).
        kTF96c = keep.tile([64, 2, LC], f8, tag="kTF96c", name="kTF96c")
        qhatF96c = keep.tile([64, 2, L], f8, tag="qhatF96c", name="qhatF96c")
        kTF96_sb = [kTF96c[32 * g : 32 * g + 32, :, :] for g in range(2)]
        qhatF96_sb = [qhatF96c[32 * g : 32 * g + 32, :, :] for g in range(2)]
        v_sb = [keep.tile([128, HG, 96], bf16, tag=f"v{mt}", name=f"v{mt}") for mt in range(MT)]
        on2_sb = [keep.tile([128, L], bf16, tag=f"on2_{j}", name=f"on2_{j}") for j in range(HG // 2)]

        vpool = persist.enter_context(tc.tile_pool(name="vpool", bufs=1))
        ebpool = persist.enter_context(tc.tile_pool(name="ebpool", bufs=4))
        ebt_tiles = {}
        HMT = MT // 4  # mts per ebt quarter-tile

        def load_ebt(h, half):
            t = ebpool.tile([128, HMT, L], bf16, tag="expb", name=f"ebt{h}_{half}")
            lo = half * HMT
            dma.dma_start(
                out=t,
                in_=expbT[h, lo * 128 : (lo + HMT) * 128, :].rearrange(
                    "(g p) l -> p g l", p=128
                ),
            )
            ebt_tiles[(h, half)] = t

        # ---------------- phase 1: projections + norms ----------------
        with ExitStack() as p1:
            wpool = p1.enter_context(tc.tile_pool(name="wpool", bufs=1))
            apool = p1.enter_context(tc.tile_pool(name="apool", bufs=1))
            spool = p1.enter_context(tc.tile_pool(name="spool", bufs=1))
            psA = p1.enter_context(tc.tile_pool(name="psA", bufs=3, space="PSUM"))

            # q-phase inputs first so PE can start ASAP, then k/v inputs
            wq_t = wpool.tile([128, KT, OC], f8, tag="wq")
            wqT_r = wqT.rearrange("(t p) o -> p t o", p=128)
            dma.dma_start(out=wq_t, in_=wqT_r)
            x_t = apool.tile([128, KT, L], f8, tag="x")
            xT_r = xT8.rearrange("(t p) l -> p t l", p=128)
            dma.dma_start(out=x_t, in_=xT_r)
            wk_t = wpool.tile([128, KT, OC], f8, tag="wk")
            dma.dma_start(out=wk_t, in_=wkT.rearrange("(t p) o -> p t o", p=128))
            ctx8_t = apool.tile([128, KT, LC], f8, tag="ctx8")
            dma.dma_start(out=ctx8_t, in_=ctxT8.rearrange("(t p) m -> p t m", p=128))
            hsum_t = wpool.tile([128, OCT, HG], bf16, tag="hsum")
            dma.dma_start(out=hsum_t, in_=hsum.rearrange("(t p) h -> p t h", p=128))
            hsum_sb = [hsum_t[:, ot, :] for ot in range(OCT)]
            hbc_sb = wpool.tile([HG, OC], bf16, tag="hbc")
            dma.dma_start(out=hbc_sb, in_=hbc[:, :])
            sminv_sb = wpool.tile([HG, 1], f32, tag="sminv")
            dma.dma_start(out=sminv_sb, in_=sminv[:, :])
            # head-0 exp(bias) front chunk early so the bias-multiply
            # stream doesn't stall; remainder after ctx/wv
            load_ebt(0, 0)
            load_ebt(0, 2)
            load_ebt(0, 3)
            ctx_t = vpool.tile([128, KT, LC], bf16, tag="ctx")
            dma.dma_start(out=ctx_t, in_=ctxT.rearrange("(t p) m -> p t m", p=128))
            ctx_sb = [ctx_t[:, kt, :] for kt in range(KT)]
            wv_t = vpool.tile([128, KT, OC], bf16, tag="wv")
            dma.dma_start(out=wv_t, in_=wvT.rearrange("(t p) o -> p t o", p=128))
            wv_sb = [wv_t[:, kt, :] for kt in range(KT)]
            load_ebt(0, 1)

            # q projection (fp8 DoubleRow): qT (bf16) and q^2 (bf16)
            qT_sb, q2_sb = [], []
            for ot in range(OCT):
                ps = psA.tile([128, L], f32, tag="psA")
                oc_sl = slice(ot * 128, (ot + 1) * 128)
                for kt in range(0, KT, 2):
                    for nch in range(L // 512):
                        nsl = slice(nch * 512, (nch + 1) * 512)
                        nc.tensor.matmul(
                            ps[:, nsl],
                            wq_t[:, kt : kt + 2, oc_sl],
                            x_t[:, kt : kt + 2, nsl],
                            start=(kt == 0),
                            stop=(kt == KT - 2),
                            perf_mode=PM.DoubleRow,
                        )
                t = apool.tile([128, L], bf16, tag=f"qT{ot}")
                nc.vector.tensor_copy(t, ps)
                qT_sb.append(t)
                t2 = apool.tile([128, L], bf16, tag=f"q2{ot}")
                nc.vector.tensor_mul(t2, t, t)
                q2_sb.append(t2)

            # k projection (fp8 DoubleRow, two Lc halves per oc-tile)
            # + k row norms; evacuate straight to fp8 kTF layout
            n2k_all = spool.tile([128, OCT], f32, tag="n2k_all")
            rsk_all = spool.tile([128, OCT], f32, tag="rsk_all")
            lnk_all = spool.tile([128, OCT], f32, tag="lnk_all")

            def k_proj(ot):
                g, sub = ot // 2, ot % 2
                oc_sl = slice(ot * 128, (ot + 1) * 128)
                n2kh = spool.tile([128, 2], f32, tag=f"n2kh{ot}")
                for half in range(2):
                    ps = psA.tile([128, 1024], f32, tag="psA")
                    for kt in range(0, KT, 2):
                        for nch in range(2):
                            gsl = slice(
                                half * 1024 + nch * 512, half * 1024 + (nch + 1) * 512
                            )
                            nsl = slice(nch * 512, (nch + 1) * 512)
                            nc.tensor.matmul(
                                ps[:, nsl],
                                wk_t[:, kt : kt + 2, oc_sl],
                                ctx8_t[:, kt : kt + 2, gsl],
                                start=(kt == 0),
                                stop=(kt == KT - 2),
                                perf_mode=PM.DoubleRow,
                            )
                    kt_half = kTF_sb[g][:, sub, half * 1024 : (half + 1) * 1024]
                    nc.scalar.activation(kt_half, ps, AF.Copy)
                    k2s = spool.tile([128, 1024], f8, tag="k2s", bufs=2, name="k2s")
                    # k2s = kt*kt with fused row-sum accumulation (from the
                    # fp8 copy; quantization error on the norm is ~0.1%)
                    nc.vector.scalar_tensor_tensor(
                        k2s,
                        kt_half,
                        1.0,
                        kt_half,
                        op0=ALU.mult,
                        op1=ALU.mult,
                        accum_out=n2kh[:, half : half + 1],
                    )
                nc.vector.tensor_add(
                    n2k_all[:, ot : ot + 1], n2kh[:, 0:1], n2kh[:, 1:2]
                )

            # q norms: n2[h,l] -> s = R*sm/sqrt(n2) -> broadcast to rows
            with tc.tile_pool(name="psN", bufs=1, space="PSUM") as psN:
                psn2 = psN.tile([HG, L], f32, tag="psn2")
                for ot in range(OCT):
                    for nch in range(L // 512):
                        nsl = slice(nch * 512, (nch + 1) * 512)
                        nc.tensor.matmul(
                            psn2[:, nsl],
                            hsum_sb[ot],
                            q2_sb[ot][:, nsl],
                            start=(ot == 0),
                            stop=(ot == OCT - 1),
                        )
                k_proj(0)
                k_proj(1)
                k_proj(2)
                k_proj(3)
                nc.scalar.activation(lnk_all, n2k_all, AF.Ln)
                nc.scalar.activation(rsk_all, lnk_all, AF.Exp, scale=-0.5)
                t8 = spool.tile([HG, L], bf16, tag="t8")
                nc.scalar.activation(t8, psn2, AF.Ln, scale=sminv_sb[:, 0:1])
            s_sb = spool.tile([HG, L], bf16, tag="s_sb")
            nc.scalar.activation(s_sb, t8, AF.Exp, scale=-0.5)
            sbc_sb = []
            for ot in range(OCT):
                ps = psA.tile([128, L], f32, tag="psA")
                for nch in range(L // 512):
                    nsl = slice(nch * 512, (nch + 1) * 512)
                    nc.tensor.matmul(
                        ps[:, nsl],
                        hbc_sb[:, ot * 128 : (ot + 1) * 128],
                        s_sb[:, nsl],
                        start=True,
                        stop=True,
                    )
                sbc = spool.tile([128, L], bf16, tag=f"sbc{ot}", name="sbc")
                nc.vector.tensor_copy(sbc, ps)
                sbc_sb.append(sbc)

            # qhat = (qT * rsk_per_partition) * s_broadcast -> fp8 layout
            for ot in range(OCT):
                g, sub = ot // 2, ot % 2
                nc.vector.scalar_tensor_tensor(
                    qhatF_sb[g][:, sub, :],
                    qT_sb[ot],
                    rsk_all[:, ot : ot + 1],
                    sbc_sb[ot],
                    op0=ALU.mult,
                    op1=ALU.mult,
                )
            # partition-shifted copies of head 3 (base 96 -> 0)
            for g in range(2):
                dma.dma_start(out=kTF96_sb[g], in_=kTF_sb[g][96:128, :, :])
                dma.dma_start(out=qhatF96_sb[g], in_=qhatF_sb[g][96:128, :, :])
            if dbg:
                dma.dma_start(out=dbg["qhatF"][:, :, :], in_=qhatF_sb[0])
                dma.dma_start(out=dbg["kTF"][:, :, :], in_=kTF_sb[0])
                dma.dma_start(out=dbg["ssb"][:, :], in_=s_sb)
                dma.dma_start(out=dbg["rsk"][:, :], in_=rsk_all)
                dma.dma_start(out=dbg["qT0"][:, :], in_=qT_sb[0])


        # ---------------- phase 2: attention ----------------
        # Per head: S -> exp -> bias-mult m-tiles stream at the scalar
        # engine's pace (one m-tile per PSUM tile). AV matmuls trail
        # roughly one head behind via a global cursor so exp never waits
        # on PE; the v projection slots in after head 0's stream.
        with ExitStack() as p2:
            stpool = p2.enter_context(tc.tile_pool(name="stream", bufs=4))
            ypool2 = p2.enter_context(tc.tile_pool(name="ypool2", bufs=1))
            yp_box = {"ypart": ypool2.tile([128, LT, C], bf16, tag="ypart", name="ypart")}
            ptbpool = p2.enter_context(tc.tile_pool(name="ptbpool", bufs=17))
            tpool = p2.enter_context(tc.tile_pool(name="tails", bufs=1))
            psS = p2.enter_context(tc.tile_pool(name="psS", bufs=2, space="PSUM"))
            psO = p2.enter_context(tc.tile_pool(name="psO", bufs=1, space="PSUM"))

            ptb_all = {}  # (h, mt) -> ptb tile
            pso_all = {}  # h -> AV psum accumulator
            av_queue = []  # (h, mt), strictly ordered
            emitted_mts = [0] * HG
            v_state = {"done": 0, "psV": None}

            def v_proj_mt(mt):
                # v projection for one m-tile into (m, head, 1+d) layout
                # (ones column first so the AV rowsum lands at partition 0)
                ps = v_state["psV"].tile([128, OC], f32, tag="psV", name="psV")
                msl = slice(mt * 128, (mt + 1) * 128)
                for kt in range(KT):
                    nc.tensor.matmul(
                        ps,
                        ctx_sb[kt][:, msl],
                        wv_sb[kt],
                        start=(kt == 0),
                        stop=(kt == KT - 1),
                    )
                nc.scalar.activation(
                    v_sb[mt][:, :, 32:96],
                    ps.rearrange("p (h d) -> p h d", h=HG),
                    AF.Copy,
                )
                # col 0 = ones (rowsum lands at pso partition 0); cols
                # 1-31 are dead padding so values start at base 32.
                nc.vector.memset(v_sb[mt][:, :, 0:1], 1.0)
                nc.vector.memset(v_sb[mt][:, :, 1:32], 0.0)
                if dbg and mt == 0:
                    dma.dma_start(out=dbg["v0"][:, :, :], in_=v_sb[0])
                v_state["done"] = mt + 1

            tails_done = [0]

            def emit_tail(h):
                tails_done[0] += 1
                pso = pso_all.pop(h)
                # evacuate pso right away so its PSUM banks free for the
                # next head; tail math runs from SBUF.
                osb = tpool.tile([96, L], f32, tag="osb", bufs=2, name="osb")
                nc.vector.tensor_copy(osb, pso)
                # rowsum sits at partition 0 (ones column is v[..., 0]);
                # broadcast it and divide on the idle Pool engine.
                rrec = tpool.tile([1, L], bf16, tag="rrec", bufs=1, name="rrec")
                with nc.allow_low_precision(reason="bf16 rowsum recip, ~0.2%"):
                    nc.vector.reciprocal(rrec, osb[0:1, :])
                rb = tpool.tile([96, L], bf16, tag="rb", bufs=1, name="rb")
                nc.gpsimd.partition_broadcast(rb, rrec, channels=96)
                if h % 2 == 0:
                    dst = on2_sb[h // 2][0:D, :]
                else:
                    dst = tpool.tile([D, L], bf16, tag="onodd", bufs=1, name="onodd")
                # base-32 APs may touch at most 32 partitions: split halves
                eng = nc.vector if h == 6 else nc.gpsimd
                eng.tensor_mul(dst[0:32, :], osb[32:64, :], rb[32:64, :])
                eng.tensor_mul(dst[32:64, :], osb[64:96, :], rb[64:96, :])
                if h % 2 == 1:
                    dma.dma_start(out=on2_sb[h // 2][D:128, :], in_=dst)
                if dbg and h == 0:
                    dma.dma_start(out=dbg["osb"][:, :], in_=osb)
                if dbg and h == 1:
                    dma.dma_start(out=dbg["on2"][:, :], in_=on2_sb[0])

            def emit_avs(budget):
                while budget > 0 and av_queue:
                    h, mt = av_queue[0]
                    if emitted_mts[h] <= mt or v_state["done"] <= mt:
                        return
                    if h not in pso_all:
                        pso_all[h] = psO.tile([96, L], f32, tag="pso", name="pso")
                    pso = pso_all[h]
                    ptb_tile = ptb_all.pop((h, mt))
                    for nch in range(2):
                        nsl = slice(nch * 512, (nch + 1) * 512)
                        nc.tensor.matmul(
                            pso[:, nsl],
                            v_sb[mt][:, h, :],
                            ptb_tile[:, nsl],
                            start=(mt == 0),
                            stop=(mt == MT - 1),
                        )
                    av_queue.pop(0)
                    budget -= 1
                    if mt == MT - 1:
                        emit_tail(h)

            psYa_box = {"pool": None, "lt": 0}
            v_state["psVcm"] = tc.tile_pool(name="psV", bufs=2, space="PSUM")
            v_state["psV"] = v_state["psVcm"].__enter__()

            def emit_ppass1(lt):
                # first half of the output projection (heads 0..3) into
                # ypart while later heads stream; phase 3 adds j=2,3.
                lsl = slice(lt * 128, (lt + 1) * 128)
                psy = psYa_box["pool"].tile(
                    [128, C], f32, tag="psya", name="psya", bufs=1
                )
                for j in range(2):
                    for nch in range(C // 512):
                        nsl = slice(nch * 512, (nch + 1) * 512)
                        nc.tensor.matmul(
                            psy[:, nsl],
                            on2_sb[j][:, lsl],
                            wp2_sb[j][:, nsl],
                            start=(j == 0),
                            stop=(j == 1),
                        )
                nc.vector.tensor_copy(yp_box["ypart"][:, lt, :], psy)

            HEAD_ORDER = [0, 1, 2, 3, 4, 5, 7, 6]
            for hidx, hh in enumerate(HEAD_ORDER):
                nxt = HEAD_ORDER[hidx + 1] if hidx + 1 < HG else None
                g, h4 = hh // 4, hh % 4
                psl = slice(32 * h4, 32 * h4 + 32)
                if h4 == 3:
                    k_src, q_src = kTF96_sb[g], qhatF96_sb[g]
                else:
                    k_src = kTF_sb[g][psl, :, :]
                    q_src = qhatF_sb[g][psl, :, :]
                if hh == 4:
                    psYa_box["pool"] = p2.enter_context(
                        tc.tile_pool(name="psYa", bufs=1, space="PSUM")
                    )

                if hh == 1:
                    # wp is only read by the output projection
                    dma.dma_start(
                        out=wp_t, in_=wpT.rearrange("(j p) o -> p j o", p=128)
                    )
                for mt in range(MT):
                    # prefetch next head's quarter q-1 only after this
                    # head's quarter q-1 is fully consumed (buffer reuse)
                    if mt % HMT == 0 and mt > 0 and nxt is not None:
                        load_ebt(nxt, mt // HMT - 1)
                    msl = slice(mt * 128, (mt + 1) * 128)
                    pss = psS.tile([128, L], f32, tag="pss", name="pss")
                    for nch in range(2):
                        nsl = slice(nch * 512, (nch + 1) * 512)
                        nc.tensor.matmul(
                            pss[:, nsl],
                            k_src[:, :, msl],
                            q_src[:, :, nsl],
                            start=True,
                            stop=True,
                            perf_mode=PM.DoubleRow,
                        )
                    praw = stpool.tile([128, L], bf16, tag="praw", name="praw")
                    nc.scalar.activation(praw, pss, AF.Exp, scale=1.0 / R)
                    n_live = len(ptb_all) + 1
                    assert n_live <= 17, f"ptb live {n_live} exceeds pool bufs"
                    ptb = ptbpool.tile([128, L], bf16, tag="ptb", name="ptb")
                    nc.vector.tensor_mul(
                        ptb, praw, ebt_tiles[(hh, mt // HMT)][:, mt % HMT, :]
                    )
                    if mt % HMT == HMT - 1:
                        ebt_tiles.pop((hh, mt // HMT))
                    if dbg and hh == 0 and mt == 0:
                        dma.dma_start(out=dbg["praw"][:, :], in_=praw)
                        dma.dma_start(out=dbg["ptb"][:, :], in_=ptb)
                    ptb_all[(hh, mt)] = ptb
                    emitted_mts[hh] = mt + 1
                    av_queue.append((hh, mt))
                    if hh == 0 and mt >= 6:
                        v_proj_mt(mt - 6)
                    if hh == 1 and 10 + mt // 2 < MT and mt % 2 == 0:
                        v_proj_mt(10 + mt // 2)
                    if (
                        psYa_box["lt"] < LT
                        and psYa_box["pool"] is not None
                        and tails_done[0] >= 4
                        and (mt % 8 == 4)
                    ):
                        emit_ppass1(psYa_box["lt"])
                        psYa_box["lt"] += 1
                    emit_avs(3 if len(av_queue) > 12 else (2 if len(av_queue) > 6 or hidx == HG - 1 else 1))

                if nxt is not None:
                    load_ebt(nxt, 3)
                if hh == 1:
                    v_state["psVcm"].__exit__(None, None, None)

            while av_queue:
                emit_avs(1000)
            while psYa_box["lt"] < LT:
                emit_ppass1(psYa_box["lt"])
                psYa_box["lt"] += 1

            # ---------------- phase 3: output projection ----------------
            ypool = p2.enter_context(tc.tile_pool(name="ypool", bufs=2))

            for lt in range(LT):
                lsl = slice(lt * 128, (lt + 1) * 128)
                ysb = ypool.tile([128, C], bf16, tag="ysb")
                psy = psS.tile([128, L], f32, tag="pss", name="pss")
                for j in range(2, HG // 2):
                    for nch in range(C // 512):
                        nsl = slice(nch * 512, (nch + 1) * 512)
                        nc.tensor.matmul(
                            psy[:, nsl],
                            on2_sb[j][:, lsl],
                            wp2_sb[j][:, nsl],
                            start=(j == 2),
                            stop=(j == HG // 2 - 1),
                        )
                nc.vector.tensor_add(ysb, psy, yp_box["ypart"][:, lt, :])
                dma.dma_start(out=y[lsl, :], in_=ysb)

    nc.compile()
    return nc


def _get_nc():
    if "nc" not in _NC_CACHE:
        _NC_CACHE["nc"] = build_nc()
    return _NC_CACHE["nc"]


def _col_perm():
    """New column order within a core's OC block: tile t in 0..3 holds
    heads 4*(t//2)..4*(t//2)+3, head-dim lanes 32*(t%2)..32*(t%2)+31;
    partition p maps to (h = 4*(t//2) + p//32, d = 32*(t%2) + p%32).
    Returns perm with perm[t*128 + p] = old column index h*64 + d."""
    perm = np.empty(OC, dtype=np.int64)
    for t in range(4):
        for p in range(128):
            h = 4 * (t // 2) + p // 32
            d = 32 * (t % 2) + p % 32
            perm[t * 128 + p] = h * 64 + d
    return perm


def kernel(x, context, attn_bias, Wq, Wk, Wv, Wp, bp, scale_mul):
    global LAST_RESULT
    x = np.asarray(x, dtype=np.float32)
    context = np.asarray(context, dtype=np.float32)
    attn_bias = np.asarray(attn_bias, dtype=np.float32)
    Wq = np.asarray(Wq, dtype=np.float32)
    Wk = np.asarray(Wk, dtype=np.float32)
    Wv = np.asarray(Wv, dtype=np.float32)
    Wp = np.asarray(Wp, dtype=np.float32)
    bp = np.asarray(bp, dtype=np.float32)
    scale_mul = np.asarray(scale_mul, dtype=np.float32)

    sm = np.exp(np.minimum(scale_mul, MAX_SCALE_MUL)).reshape(H)  # (H,)
    expb = np.exp(attn_bias[0])  # (H, L, Lc)

    perm = _col_perm()
    # hsum/hbc map partitions of tile t to heads under the new order
    hsum = np.zeros((OC, HG), dtype=BF16)
    hbc = np.zeros((HG, OC), dtype=BF16)
    for t in range(4):
        for p in range(128):
            h = 4 * (t // 2) + p // 32
            hsum[t * 128 + p, h] = 1.0
            hbc[h, t * 128 + p] = 1.0

    gshard = {}
    for g in range(2):
        rows = slice(g * OC, (g + 1) * OC)
        heads = slice(g * HG, (g + 1) * HG)
        wq_g = (WSCALE * Wq[rows, :])[perm, :]  # [OC, C] reordered rows
        wk_g = (WSCALE * Wk[rows, :])[perm, :]
        gshard[g] = dict(
            wqT=np.ascontiguousarray(wq_g.T).astype(F8),
            wkT=np.ascontiguousarray(wk_g.T).astype(F8),
            wvT=np.ascontiguousarray(Wv[rows, :].T).astype(BF16),
            wpT=np.ascontiguousarray(Wp[:, rows].T).astype(BF16),
            expbT=np.ascontiguousarray(
                np.transpose(expb[heads], (0, 2, 1))
            ).astype(BF16),
            # s = (psn2 * sminv)^-0.5 should equal R*sm/sqrt(psn2)
            sminv=(1.0 / (R * R * sm[heads] ** 2)).reshape(HG, 1).astype(np.float32),
        )
    bshard = {}
    for b in range(B):
        xb = np.ascontiguousarray(x[b].T)
        cb = np.ascontiguousarray(context[b].T)
        bshard[b] = dict(
            xT8=xb.astype(F8),
            ctxT=cb.astype(BF16),
            ctxT8=cb.astype(F8),
        )

    in_maps = []
    for core in range(N_CORES):
        b, g = core // 2, core % 2
        m = dict(hsum=hsum, hbc=hbc)
        m.update(gshard[g])
        m.update(bshard[b])
        in_maps.append(m)

    nc = _get_nc()
    res = run_bass_kernel_spmd(
        nc, in_maps, core_ids=list(range(N_CORES)), trace=TRACE
    )
    LAST_RESULT = res
    outs = [r["y"].astype(np.float32) for r in res.results]
    out = np.stack(
        [outs[2 * b] + outs[2 * b + 1] + bp[None, :] for b in range(B)]
    ).astype(np.float32)
    return out


# revision 76
# speedup vs baseline: 1.0094x; 1.0094x over previous
"""Distributed Trainium2 Bass kernel for nn_CrossAttention (B=4, L=1024,
Lc=2048, C=1024, H=16).

Sharding: 8 cores = 4 batches x 2 head-groups of 8 heads. Each core
computes its batch's q/k/v projections for its 8 heads, the attention,
and a partial output projection (row-shard of Wp). Host sums the two
partial outputs per batch (bf16) and adds bp.

Precision: q/k projections and the S=q@k^T matmul run in fp8e4 with
perf_mode=DoubleRow (two contraction subtiles per instruction -> 2x PE
throughput, K=256 per instruction for the projections). Wq/Wk are
pre-scaled by 32 on the host (q/k are l2-normalized downstream so the
scale cancels); qhat is rescaled by R=2048 into fp8 range and exp()
compensates with scale=1/R. Weight columns are reordered host-side so
each 128-partition tile holds 4 heads x 32 head-dim lanes, giving the
S matmul its [32, 2, *] DoubleRow layout directly. V/AV/P-projection
stay bf16 for accuracy.

Schedule: per head, S -> exp -> bias-multiply m-tiles stream at the
scalar engine's pace (exp is the global bottleneck at ~134us busy).
AV matmuls trail roughly one head behind via a global cursor so exp
never stalls on PE; the v projection interleaves into heads 0-1; the
first half of the output projection (head pairs j=0,1) runs in-stream
into an SBUF partial (ypart) once heads 0-3 finish; heads run in order
[0,1,2,3,4,5,7,6] so the final tail avoids the cross-partition DMA.
The softmax denominator rides as a leading ones-column in V (rowsum
lands at PSUM partition 0 for partition_broadcast); values sit at
partitions 32..95 to satisfy base-partition rules.
"""

import os
import sys
from contextlib import ExitStack

sys.path.insert(0, "/opt/trn_rl_repo")

import numpy as np
import ml_dtypes

import concourse.bass as bass
from concourse import bacc
import concourse.mybir as mybir
import concourse.tile as tile
from concourse.bass_utils import run_bass_kernel_spmd

BF16 = ml_dtypes.bfloat16
F8 = ml_dtypes.float8_e4m3
AF = mybir.ActivationFunctionType
ALU = mybir.AluOpType
AX = mybir.AxisListType
PM = mybir.MatmulPerfMode

# All ACT functions used here (Copy/Exp/Ln) live in the
# natural_log_exp_and_others table set; blank the other sets so
# insert_act_table_loads emits exactly one table load.
from concourse.hw_specs import get_activation_tables as _gat_orig


def _gat_one_set(arch):
    t = _gat_orig(arch)
    return {
        k: (v if k == "natural_log_exp_and_others" else set()) for k, v in t.items()
    }


bacc.get_activation_tables = _gat_one_set

B, L, LC, C, H = 4, 1024, 2048, 1024, 16
HG = 8  # heads per core
D = 64  # head dim
OC = HG * D  # 512 output channels per core
N_CORES = 8
MAX_SCALE_MUL = float(np.log(100.0))
WSCALE = 32.0  # host pre-scale on Wq/Wk before fp8 quantization
R = 2048.0  # qhat rescale into fp8 range; exp() applies 1/R

# module-level knobs for test harness
TRACE = False
LAST_RESULT = None

_NC_CACHE = {}


def build_nc():
    f32, bf16, f8 = mybir.dt.float32, mybir.dt.bfloat16, mybir.dt.float8e4
    nc = bacc.Bacc()

    xT8 = nc.declare_dram_parameter("xT8", [C, L], f8, isOutput=False)
    ctxT = nc.declare_dram_parameter("ctxT", [C, LC], bf16, isOutput=False)
    ctxT8 = nc.declare_dram_parameter("ctxT8", [C, LC], f8, isOutput=False)
    wqT = nc.declare_dram_parameter("wqT", [C, OC], f8, isOutput=False)
    wkT = nc.declare_dram_parameter("wkT", [C, OC], f8, isOutput=False)
    wvT = nc.declare_dram_parameter("wvT", [C, OC], bf16, isOutput=False)
    wpT = nc.declare_dram_parameter("wpT", [OC, C], bf16, isOutput=False)
    expbT = nc.declare_dram_parameter("expbT", [HG, LC, L], bf16, isOutput=False)
    hsum = nc.declare_dram_parameter("hsum", [OC, HG], bf16, isOutput=False)
    hbc = nc.declare_dram_parameter("hbc", [HG, OC], bf16, isOutput=False)
    sminv = nc.declare_dram_parameter("sminv", [HG, 1], f32, isOutput=False)
    y = nc.declare_dram_parameter("y", [L, C], bf16, isOutput=True)
    dbg = {}
    if os.environ.get("KDBG", "0") == "1":
        dbg["qhatF"] = nc.declare_dram_parameter("d_qhatF", [128, 2, L], f8, isOutput=True)
        dbg["kTF"] = nc.declare_dram_parameter("d_kTF", [128, 2, LC], f8, isOutput=True)
        dbg["ssb"] = nc.declare_dram_parameter("d_ssb", [HG, L], bf16, isOutput=True)
        dbg["rsk"] = nc.declare_dram_parameter("d_rsk", [128, 4], f32, isOutput=True)
        dbg["qT0"] = nc.declare_dram_parameter("d_qT0", [128, L], bf16, isOutput=True)
        dbg["v0"] = nc.declare_dram_parameter("d_v0", [128, HG, 96], bf16, isOutput=True)
        dbg["praw"] = nc.declare_dram_parameter("d_praw", [128, L], bf16, isOutput=True)
        dbg["ptb"] = nc.declare_dram_parameter("d_ptb", [128, L], bf16, isOutput=True)
        dbg["osb"] = nc.declare_dram_parameter("d_osb", [96, L], f32, isOutput=True)
        dbg["on2"] = nc.declare_dram_parameter("d_on2", [128, L], bf16, isOutput=True)

    KT = C // 128  # 8 contraction tiles (DoubleRow: pairs -> 4 steps)
    OCT = OC // 128  # 4 output-channel tiles
    MT = LC // 128  # 16 context tiles
    LT = L // 128  # 8 query tiles

    with tile.TileContext(nc) as tc, ExitStack() as persist:
        keep = persist.enter_context(tc.tile_pool(name="keep", bufs=1))
        dma = nc.sync

        # head PAIRS stacked across the 128 partitions for the output
        # projection: contraction becomes standard K=128 matmuls
        wp_t = keep.tile([128, HG // 2, C], bf16, tag="wp")
        wp2_sb = [wp_t[:, j, :] for j in range(HG // 2)]

        # fp8 DoubleRow layouts for the S matmul:
        # group g in {0,1} holds heads 4g..4g+3; head h4 = partitions
        # 32*h4..32*h4+31, subtile i covers head-dim 32i..32i+31.
        kTF_sb = [keep.tile([128, 2, LC], f8, tag=f"kTF{g}", name=f"kTF{g}") for g in range(2)]
        qhatF_sb = [keep.tile([128, 2, L], f8, tag=f"qhatF{g}", name=f"qhatF{g}") for g in range(2)]
        # matmul operands must have base partition in {0,32,64}; head 3 of
        # each group lives at base 96, so keep partition-shifted copies
        # (group g at base 32*g of one shared tile).
        kTF96c = keep.tile([64, 2, LC], f8, tag="kTF96c", name="kTF96c")
        qhatF96c = keep.tile([64, 2, L], f8, tag="qhatF96c", name="qhatF96c")
        kTF96_sb = [kTF96c[32 * g : 32 * g + 32, :, :] for g in range(2)]
        qhatF96_sb = [qhatF96c[32 * g : 32 * g + 32, :, :] for g in range(2)]
        v_sb = [keep.tile([128, HG, 96], bf16, tag=f"v{mt}", name=f"v{mt}") for mt in range(MT)]
        on2_sb = [keep.tile([128, L], bf16, tag=f"on2_{j}", name=f"on2_{j}") for j in range(HG // 2)]

        vpool = persist.enter_context(tc.tile_pool(name="vpool", bufs=1))
        ebpool = persist.enter_context(tc.tile_pool(name="ebpool", bufs=4))
        ebt_tiles = {}
        HMT = MT // 4  # mts per ebt quarter-tile

        def load_ebt(h, half):
            t = ebpool.tile([128, HMT, L], bf16, tag="expb", name=f"ebt{h}_{half}")
            lo = half * HMT
            dma.dma_start(
                out=t,
                in_=expbT[h, lo * 128 : (lo + HMT) * 128, :].rearrange(
                    "(g p) l -> p g l", p=128
                ),
            )
            ebt_tiles[(h, half)] = t

        # ---------------- phase 1: projections + norms ----------------
        with ExitStack() as p1:
            wpool = p1.enter_context(tc.tile_pool(name="wpool", bufs=1))
            apool = p1.enter_context(tc.tile_pool(name="apool", bufs=1))
            spool = p1.enter_context(tc.tile_pool(name="spool", bufs=1))
            psA = p1.enter_context(tc.tile_pool(name="psA", bufs=3, space="PSUM"))

            # q-phase inputs first so PE can start ASAP, then k/v inputs
            wq_t = wpool.tile([128, KT, OC], f8, tag="wq")
            wqT_r = wqT.rearrange("(t p) o -> p t o", p=128)
            dma.dma_start(out=wq_t, in_=wqT_r)
            x_t = apool.tile([128, KT, L], f8, tag="x")
            xT_r = xT8.rearrange("(t p) l -> p t l", p=128)
            dma.dma_start(out=x_t, in_=xT_r)
            wk_t = wpool.tile([128, KT, OC], f8, tag="wk")
            dma.dma_start(out=wk_t, in_=wkT.rearrange("(t p) o -> p t o", p=128))
            ctx8_t = apool.tile([128, KT, LC], f8, tag="ctx8")
            dma.dma_start(out=ctx8_t, in_=ctxT8.rearrange("(t p) m -> p t m", p=128))
            hsum_t = wpool.tile([128, OCT, HG], bf16, tag="hsum")
            dma.dma_start(out=hsum_t, in_=hsum.rearrange("(t p) h -> p t h", p=128))
            hsum_sb = [hsum_t[:, ot, :] for ot in range(OCT)]
            hbc_sb = wpool.tile([HG, OC], bf16, tag="hbc")
            dma.dma_start(out=hbc_sb, in_=hbc[:, :])
            sminv_sb = wpool.tile([HG, 1], f32, tag="sminv")
            dma.dma_start(out=sminv_sb, in_=sminv[:, :])
            # head-0 exp(bias) front chunk early so the bias-multiply
            # stream doesn't stall; remainder after ctx/wv
            load_ebt(0, 0)
            load_ebt(0, 2)
            load_ebt(0, 3)
            ctx_t = vpool.tile([128, KT, LC], bf16, tag="ctx")
            dma.dma_start(out=ctx_t, in_=ctxT.rearrange("(t p) m -> p t m", p=128))
            ctx_sb = [ctx_t[:, kt, :] for kt in range(KT)]
            wv_t = vpool.tile([128, KT, OC], bf16, tag="wv")
            dma.dma_start(out=wv_t, in_=wvT.rearrange("(t p) o -> p t o", p=128))
            wv_sb = [wv_t[:, kt, :] for kt in range(KT)]
            load_ebt(0, 1)

            # q projection (fp8 DoubleRow): qT (bf16) and q^2 (bf16)
            qT_sb, q2_sb = [], []
            for ot in range(OCT):
                ps = psA.tile([128, L], f32, tag="psA")
                oc_sl = slice(ot * 128, (ot + 1) * 128)
                for kt in range(0, KT, 2):
                    for nch in range(L // 512):
                        nsl = slice(nch * 512, (nch + 1) * 512)
                        nc.tensor.matmul(
                            ps[:, nsl],
                            wq_t[:, kt : kt + 2, oc_sl],
                            x_t[:, kt : kt + 2, nsl],
                            start=(kt == 0),
                            stop=(kt == KT - 2),
                            perf_mode=PM.DoubleRow,
                        )
                t = apool.tile([128, L], bf16, tag=f"qT{ot}")
                nc.vector.tensor_copy(t, ps)
                qT_sb.append(t)
                t2 = apool.tile([128, L], bf16, tag=f"q2{ot}")
                nc.vector.tensor_mul(t2, t, t)
                q2_sb.append(t2)

            # k projection (fp8 DoubleRow, two Lc halves per oc-tile)
            # + k row norms; evacuate straight to fp8 kTF layout
            n2k_all = spool.tile([128, OCT], f32, tag="n2k_all")
            rsk_all = spool.tile([128, OCT], f32, tag="rsk_all")
            lnk_all = spool.tile([128, OCT], f32, tag="lnk_all")

            def k_proj(ot):
                g, sub = ot // 2, ot % 2
                oc_sl = slice(ot * 128, (ot + 1) * 128)
                n2kh = spool.tile([128, 2], f32, tag=f"n2kh{ot}")
                for half in range(2):
                    ps = psA.tile([128, 1024], f32, tag="psA")
                    for kt in range(0, KT, 2):
                        for nch in range(2):
                            gsl = slice(
                                half * 1024 + nch * 512, half * 1024 + (nch + 1) * 512
                            )
                            nsl = slice(nch * 512, (nch + 1) * 512)
                            nc.tensor.matmul(
                                ps[:, nsl],
                                wk_t[:, kt : kt + 2, oc_sl],
                                ctx8_t[:, kt : kt + 2, gsl],
                                start=(kt == 0),
                                stop=(kt == KT - 2),
                                perf_mode=PM.DoubleRow,
                            )
                    kt_half = kTF_sb[g][:, sub, half * 1024 : (half + 1) * 1024]
                    nc.scalar.activation(kt_half, ps, AF.Copy)
                    k2s = spool.tile([128, 1024], f8, tag="k2s", bufs=2, name="k2s")
                    # k2s = kt*kt with fused row-sum accumulation (from the
                    # fp8 copy; quantization error on the norm is ~0.1%)
                    nc.vector.scalar_tensor_tensor(
                        k2s,
                        kt_half,
                        1.0,
                        kt_half,
                        op0=ALU.mult,
                        op1=ALU.mult,
                        accum_out=n2kh[:, half : half + 1],
                    )
                nc.vector.tensor_add(
                    n2k_all[:, ot : ot + 1], n2kh[:, 0:1], n2kh[:, 1:2]
                )

            # q norms: n2[h,l] -> s = R*sm/sqrt(n2) -> broadcast to rows
            with tc.tile_pool(name="psN", bufs=1, space="PSUM") as psN:
                psn2 = psN.tile([HG, L], f32, tag="psn2")
                for ot in range(OCT):
                    for nch in range(L // 512):
                        nsl = slice(nch * 512, (nch + 1) * 512)
                        nc.tensor.matmul(
                            psn2[:, nsl],
                            hsum_sb[ot],
                            q2_sb[ot][:, nsl],
                            start=(ot == 0),
                            stop=(ot == OCT - 1),
                        )
                k_proj(0)
                k_proj(1)
                k_proj(2)
                k_proj(3)
                nc.scalar.activation(lnk_all, n2k_all, AF.Ln)
                nc.scalar.activation(rsk_all, lnk_all, AF.Exp, scale=-0.5)
                t8 = spool.tile([HG, L], bf16, tag="t8")
                nc.scalar.activation(t8, psn2, AF.Ln, scale=sminv_sb[:, 0:1])
            s_sb = spool.tile([HG, L], bf16, tag="s_sb")
            nc.scalar.activation(s_sb, t8, AF.Exp, scale=-0.5)
            sbc_sb = []
            for ot in range(OCT):
                ps = psA.tile([128, L], f32, tag="psA")
                for nch in range(L // 512):
                    nsl = slice(nch * 512, (nch + 1) * 512)
                    nc.tensor.matmul(
                        ps[:, nsl],
                        hbc_sb[:, ot * 128 : (ot + 1) * 128],
                        s_sb[:, nsl],
                        start=True,
                        stop=True,
                    )
                sbc = spool.tile([128, L], bf16, tag=f"sbc{ot}", name="sbc")
                nc.vector.tensor_copy(sbc, ps)
                sbc_sb.append(sbc)

            # qhat = (qT * rsk_per_partition) * s_broadcast -> fp8 layout
            for ot in range(OCT):
                g, sub = ot // 2, ot % 2
                nc.vector.scalar_tensor_tensor(
                    qhatF_sb[g][:, sub, :],
                    qT_sb[ot],
                    rsk_all[:, ot : ot + 1],
                    sbc_sb[ot],
                    op0=ALU.mult,
                    op1=ALU.mult,
                )
            # partition-shifted copies of head 3 (base 96 -> 0)
            for g in range(2):
                dma.dma_start(out=kTF96_sb[g], in_=kTF_sb[g][96:128, :, :])
                dma.dma_start(out=qhatF96_sb[g], in_=qhatF_sb[g][96:128, :, :])
            if dbg:
                dma.dma_start(out=dbg["qhatF"][:, :, :], in_=qhatF_sb[0])
                dma.dma_start(out=dbg["kTF"][:, :, :], in_=kTF_sb[0])
                dma.dma_start(out=dbg["ssb"][:, :], in_=s_sb)
                dma.dma_start(out=dbg["rsk"][:, :], in_=rsk_all)
                dma.dma_start(out=dbg["qT0"][:, :], in_=qT_sb[0])


        # ---------------- phase 2: attention ----------------
        # Per head: S -> exp -> bias-mult m-tiles stream at the scalar
        # engine's pace (one m-tile per PSUM tile). AV matmuls trail
        # roughly one head behind via a global cursor so exp never waits
        # on PE; the v projection slots in after head 0's stream.
        with ExitStack() as p2:
            stpool = p2.enter_context(tc.tile_pool(name="stream", bufs=4))
            ypool2 = p2.enter_context(tc.tile_pool(name="ypool2", bufs=1))
            yp_box = {"ypart": ypool2.tile([128, LT, C], bf16, tag="ypart", name="ypart")}
            ptbpool = p2.enter_context(tc.tile_pool(name="ptbpool", bufs=17))
            tpool = p2.enter_context(tc.tile_pool(name="tails", bufs=1))
            psS = p2.enter_context(tc.tile_pool(name="psS", bufs=2, space="PSUM"))
            psO = p2.enter_context(tc.tile_pool(name="psO", bufs=1, space="PSUM"))

            ptb_all = {}  # (h, mt) -> ptb tile
            pso_all = {}  # h -> AV psum accumulator
            av_queue = []  # (h, mt), strictly ordered
            emitted_mts = [0] * HG
            v_state = {"done": 0, "psV": None}

            def v_proj_mt(mt):
                # v projection for one m-tile into (m, head, 1+d) layout
                # (ones column first so the AV rowsum lands at partition 0)
                ps = v_state["psV"].tile([128, OC], f32, tag="psV", name="psV")
                msl = slice(mt * 128, (mt + 1) * 128)
                for kt in range(KT):
                    nc.tensor.matmul(
                        ps,
                        ctx_sb[kt][:, msl],
                        wv_sb[kt],
                        start=(kt == 0),
                        stop=(kt == KT - 1),
                    )
                nc.scalar.activation(
                    v_sb[mt][:, :, 32:96],
                    ps.rearrange("p (h d) -> p h d", h=HG),
                    AF.Copy,
                )
                # col 0 = ones (rowsum lands at pso partition 0); cols
                # 1-31 are dead padding so values start at base 32.
                nc.vector.memset(v_sb[mt][:, :, 0:1], 1.0)
                nc.vector.memset(v_sb[mt][:, :, 1:32], 0.0)
                if dbg and mt == 0:
                    dma.dma_start(out=dbg["v0"][:, :, :], in_=v_sb[0])
                v_state["done"] = mt + 1

            tails_done = [0]

            def emit_tail(h):
                tails_done[0] += 1
                pso = pso_all.pop(h)
                # evacuate pso right away so its PSUM banks free for the
                # next head; tail math runs from SBUF. The last head (6)
                # runs the chain in L-halves on DVE so the output
                # projection's first row-blocks can start sooner.
                last = h == 6
                osb = tpool.tile([96, L], f32, tag="osb", bufs=2, name="osb")
                rrec = tpool.tile([1, L], bf16, tag="rrec", bufs=1, name="rrec")
                rb = tpool.tile([96, L], bf16, tag="rb", bufs=1, name="rb")
                if h % 2 == 0:
                    dst = on2_sb[h // 2][0:D, :]
                else:
                    dst = tpool.tile([D, L], bf16, tag="onodd", bufs=1, name="onodd")
                eng = nc.vector if last else nc.gpsimd
                for sl in ([slice(0, 512), slice(512, L)] if last else [slice(0, L)]):
                    nc.vector.tensor_copy(osb[:, sl], pso[:, sl])
                    # rowsum sits at partition 0 (ones column is v[..., 0])
                    with nc.allow_low_precision(reason="bf16 rowsum recip"):
                        nc.vector.reciprocal(rrec[:, sl], osb[0:1, sl])
                    nc.gpsimd.partition_broadcast(rb[:, sl], rrec[:, sl], channels=96)
                    # base-32 APs may touch at most 32 partitions: halves
                    eng.tensor_mul(dst[0:32, sl], osb[32:64, sl], rb[32:64, sl])
                    eng.tensor_mul(dst[32:64, sl], osb[64:96, sl], rb[64:96, sl])
                if h % 2 == 1:
                    dma.dma_start(out=on2_sb[h // 2][D:128, :], in_=dst)
                if dbg and h == 0:
                    dma.dma_start(out=dbg["osb"][:, :], in_=osb)
                if dbg and h == 1:
                    dma.dma_start(out=dbg["on2"][:, :], in_=on2_sb[0])

            def emit_avs(budget):
                while budget > 0 and av_queue:
                    h, mt = av_queue[0]
                    if emitted_mts[h] <= mt or v_state["done"] <= mt:
                        return
                    if h not in pso_all:
                        pso_all[h] = psO.tile([96, L], f32, tag="pso", name="pso")
                    pso = pso_all[h]
                    ptb_tile = ptb_all.pop((h, mt))
                    for nch in range(2):
                        nsl = slice(nch * 512, (nch + 1) * 512)
                        nc.tensor.matmul(
                            pso[:, nsl],
                            v_sb[mt][:, h, :],
                            ptb_tile[:, nsl],
                            start=(mt == 0),
                            stop=(mt == MT - 1),
                        )
                    av_queue.pop(0)
                    budget -= 1
                    if mt == MT - 1:
                        emit_tail(h)

            psYa_box = {"pool": None, "lt": 0}
            v_state["psVcm"] = tc.tile_pool(name="psV", bufs=2, space="PSUM")
            v_state["psV"] = v_state["psVcm"].__enter__()

            def emit_ppass1(lt):
                # first half of the output projection (heads 0..3) into
                # ypart while later heads stream; phase 3 adds j=2,3.
                lsl = slice(lt * 128, (lt + 1) * 128)
                psy = psYa_box["pool"].tile(
                    [128, C], f32, tag="psya", name="psya", bufs=1
                )
                for j in range(2):
                    for nch in range(C // 512):
                        nsl = slice(nch * 512, (nch + 1) * 512)
                        nc.tensor.matmul(
                            psy[:, nsl],
                            on2_sb[j][:, lsl],
                            wp2_sb[j][:, nsl],
                            start=(j == 0),
                            stop=(j == 1),
                        )
                nc.vector.tensor_copy(yp_box["ypart"][:, lt, :], psy)

            HEAD_ORDER = [0, 1, 2, 3, 4, 5, 7, 6]
            for hidx, hh in enumerate(HEAD_ORDER):
                nxt = HEAD_ORDER[hidx + 1] if hidx + 1 < HG else None
                g, h4 = hh // 4, hh % 4
                psl = slice(32 * h4, 32 * h4 + 32)
                if h4 == 3:
                    k_src, q_src = kTF96_sb[g], qhatF96_sb[g]
                else:
                    k_src = kTF_sb[g][psl, :, :]
                    q_src = qhatF_sb[g][psl, :, :]
                if hh == 4:
                    psYa_box["pool"] = p2.enter_context(
                        tc.tile_pool(name="psYa", bufs=1, space="PSUM")
                    )

                if hh == 1:
                    # wp is only read by the output projection
                    dma.dma_start(
                        out=wp_t, in_=wpT.rearrange("(j p) o -> p j o", p=128)
                    )
                for mt in range(MT):
                    # prefetch next head's quarter q-1 only after this
                    # head's quarter q-1 is fully consumed (buffer reuse)
                    if mt % HMT == 0 and mt > 0 and nxt is not None:
                        load_ebt(nxt, mt // HMT - 1)
                    msl = slice(mt * 128, (mt + 1) * 128)
                    pss = psS.tile([128, L], f32, tag="pss", name="pss")
                    for nch in range(2):
                        nsl = slice(nch * 512, (nch + 1) * 512)
                        nc.tensor.matmul(
                            pss[:, nsl],
                            k_src[:, :, msl],
                            q_src[:, :, nsl],
                            start=True,
                            stop=True,
                            perf_mode=PM.DoubleRow,
                        )
                    praw = stpool.tile([128, L], bf16, tag="praw", name="praw")
                    nc.scalar.activation(praw, pss, AF.Exp, scale=1.0 / R)
                    n_live = len(ptb_all) + 1
                    assert n_live <= 17, f"ptb live {n_live} exceeds pool bufs"
                    ptb = ptbpool.tile([128, L], bf16, tag="ptb", name="ptb")
                    nc.vector.tensor_mul(
                        ptb, praw, ebt_tiles[(hh, mt // HMT)][:, mt % HMT, :]
                    )
                    if mt % HMT == HMT - 1:
                        ebt_tiles.pop((hh, mt // HMT))
                    if dbg and hh == 0 and mt == 0:
                        dma.dma_start(out=dbg["praw"][:, :], in_=praw)
                        dma.dma_start(out=dbg["ptb"][:, :], in_=ptb)
                    ptb_all[(hh, mt)] = ptb
                    emitted_mts[hh] = mt + 1
                    av_queue.append((hh, mt))
                    if hh == 0 and mt >= 6:
                        v_proj_mt(mt - 6)
                    if hh == 1 and 10 + mt // 2 < MT and mt % 2 == 0:
                        v_proj_mt(10 + mt // 2)
                    if (
                        psYa_box["lt"] < LT
                        and psYa_box["pool"] is not None
                        and tails_done[0] >= 4
                        and (mt % 8 == 4)
                    ):
                        emit_ppass1(psYa_box["lt"])
                        psYa_box["lt"] += 1
                    emit_avs(3 if len(av_queue) > 12 else (2 if len(av_queue) > 6 or hidx == HG - 1 else 1))

                if nxt is not None:
                    load_ebt(nxt, 3)
                if hh == 1:
                    v_state["psVcm"].__exit__(None, None, None)

            while av_queue:
                emit_avs(1000)
            while psYa_box["lt"] < LT:
                emit_ppass1(psYa_box["lt"])
                psYa_box["lt"] += 1

            # ---------------- phase 3: output projection ----------------
            ypool = p2.enter_context(tc.tile_pool(name="ypool", bufs=2))

            for lt in range(LT):
                lsl = slice(lt * 128, (lt + 1) * 128)
                ysb = ypool.tile([128, C], bf16, tag="ysb")
                psy = psS.tile([128, L], f32, tag="pss", name="pss")
                for j in range(2, HG // 2):
                    for nch in range(C // 512):
                        nsl = slice(nch * 512, (nch + 1) * 512)
                        nc.tensor.matmul(
                            psy[:, nsl],
                            on2_sb[j][:, lsl],
                            wp2_sb[j][:, nsl],
                            start=(j == 2),
                            stop=(j == HG // 2 - 1),
                        )
                nc.vector.tensor_add(ysb, psy, yp_box["ypart"][:, lt, :])
                dma.dma_start(out=y[lsl, :], in_=ysb)

    nc.compile()
    return nc


def _get_nc():
    if "nc" not in _NC_CACHE:
        _NC_CACHE["nc"] = build_nc()
    return _NC_CACHE["nc"]


def _col_perm():
    """New column order within a core's OC block: tile t in 0..3 holds
    heads 4*(t//2)..4*(t//2)+3, head-dim lanes 32*(t%2)..32*(t%2)+31;
    partition p maps to (h = 4*(t//2) + p//32, d = 32*(t%2) + p%32).
    Returns perm with perm[t*128 + p] = old column index h*64 + d."""
    perm = np.empty(OC, dtype=np.int64)
    for t in range(4):
        for p in range(128):
            h = 4 * (t // 2) + p // 32
            d = 32 * (t % 2) + p % 32
            perm[t * 128 + p] = h * 64 + d
    return perm


def kernel(x, context, attn_bias, Wq, Wk, Wv, Wp, bp, scale_mul):
    global LAST_RESULT
    x = np.asarray(x, dtype=np.float32)
    context = np.asarray(context, dtype=np.float32)
    attn_bias = np.asarray(attn_bias, dtype=np.float32)
    Wq = np.asarray(Wq, dtype=np.float32)
    Wk = np.asarray(Wk, dtype=np.float32)
    Wv = np.asarray(Wv, dtype=np.float32)
    Wp = np.asarray(Wp, dtype=np.float32)
    bp = np.asarray(bp, dtype=np.float32)
    scale_mul = np.asarray(scale_mul, dtype=np.float32)

    sm = np.exp(np.minimum(scale_mul, MAX_SCALE_MUL)).reshape(H)  # (H,)
    expb = np.exp(attn_bias[0])  # (H, L, Lc)

    perm = _col_perm()
    # hsum/hbc map partitions of tile t to heads under the new order
    hsum = np.zeros((OC, HG), dtype=BF16)
    hbc = np.zeros((HG, OC), dtype=BF16)
    for t in range(4):
        for p in range(128):
            h = 4 * (t // 2) + p // 32
            hsum[t * 128 + p, h] = 1.0
            hbc[h, t * 128 + p] = 1.0

    gshard = {}
    for g in range(2):
        rows = slice(g * OC, (g + 1) * OC)
        heads = slice(g * HG, (g + 1) * HG)
        wq_g = (WSCALE * Wq[rows, :])[perm, :]  # [OC, C] reordered rows
        wk_g = (WSCALE * Wk[rows, :])[perm, :]
        gshard[g] = dict(
            wqT=np.ascontiguousarray(wq_g.T).astype(F8),
            wkT=np.ascontiguousarray(wk_g.T).astype(F8),
            wvT=np.ascontiguousarray(Wv[rows, :].T).astype(BF16),
            wpT=np.ascontiguousarray(Wp[:, rows].T).astype(BF16),
            expbT=np.ascontiguousarray(
                np.transpose(expb[heads], (0, 2, 1))
            ).astype(BF16),
            # s = (psn2 * sminv)^-0.5 should equal R*sm/sqrt(psn2)
            sminv=(1.0 / (R * R * sm[heads] ** 2)).reshape(HG, 1).astype(np.float32),
        )
    bshard = {}
    for b in range(B):
        xb = np.ascontiguousarray(x[b].T)
        cb = np.ascontiguousarray(context[b].T)
        bshard[b] = dict(
            xT8=xb.astype(F8),
            ctxT=cb.astype(BF16),
            ctxT8=cb.astype(F8),
        )

    in_maps = []
    for core in range(N_CORES):
        b, g = core // 2, core % 2
        m = dict(hsum=hsum, hbc=hbc)
        m.update(gshard[g])
        m.update(bshard[b])
        in_maps.append(m)

    nc = _get_nc()
    res = run_bass_kernel_spmd(
        nc, in_maps, core_ids=list(range(N_CORES)), trace=TRACE
    )
    LAST_RESULT = res
    outs = [r["y"].astype(np.float32) for r in res.results]
    out = np.stack(
        [outs[2 * b] + outs[2 * b + 1] + bp[None, :] for b in range(B)]
    ).astype(np.float32)
    return out


# revision 77
# speedup vs baseline: 1.0114x; 1.0020x over previous
"""Distributed Trainium2 Bass kernel for nn_CrossAttention (B=4, L=1024,
Lc=2048, C=1024, H=16).

Sharding: 8 cores = 4 batches x 2 head-groups of 8 heads. Each core
computes its batch's q/k/v projections for its 8 heads, the attention,
and a partial output projection (row-shard of Wp). Host sums the two
partial outputs per batch (bf16) and adds bp.

Precision: q/k projections and the S=q@k^T matmul run in fp8e4 with
perf_mode=DoubleRow (two contraction subtiles per instruction -> 2x PE
throughput, K=256 per instruction for the projections). Wq/Wk are
pre-scaled by 32 on the host (q/k are l2-normalized downstream so the
scale cancels); qhat is rescaled by R=2048 into fp8 range and exp()
compensates with scale=1/R. Weight columns are reordered host-side so
each 128-partition tile holds 4 heads x 32 head-dim lanes, giving the
S matmul its [32, 2, *] DoubleRow layout directly. V/AV/P-projection
stay bf16 for accuracy.

Schedule: per head, S -> exp -> bias-multiply m-tiles stream at the
scalar engine's pace (exp is the global bottleneck at ~134us busy).
AV matmuls trail roughly one head behind via a global cursor so exp
never stalls on PE; the v projection interleaves into heads 0-1; the
first half of the output projection (head pairs j=0,1) runs in-stream
into an SBUF partial (ypart) once heads 0-3 finish; heads run in order
[0,1,2,3,4,5,7,6] so the final tail avoids the cross-partition DMA.
The softmax denominator rides as a leading ones-column in V (rowsum
lands at PSUM partition 0 for partition_broadcast); values sit at
partitions 32..95 to satisfy base-partition rules.
"""

import os
import sys
from contextlib import ExitStack

sys.path.insert(0, "/opt/trn_rl_repo")

import numpy as np
import ml_dtypes

import concourse.bass as bass
from concourse import bacc
import concourse.mybir as mybir
import concourse.tile as tile
from concourse.bass_utils import run_bass_kernel_spmd

BF16 = ml_dtypes.bfloat16
F8 = ml_dtypes.float8_e4m3
AF = mybir.ActivationFunctionType
ALU = mybir.AluOpType
AX = mybir.AxisListType
PM = mybir.MatmulPerfMode

# All ACT functions used here (Copy/Exp/Ln) live in the
# natural_log_exp_and_others table set; blank the other sets so
# insert_act_table_loads emits exactly one table load.
from concourse.hw_specs import get_activation_tables as _gat_orig


def _gat_one_set(arch):
    t = _gat_orig(arch)
    return {
        k: (v if k == "natural_log_exp_and_others" else set()) for k, v in t.items()
    }


bacc.get_activation_tables = _gat_one_set

B, L, LC, C, H = 4, 1024, 2048, 1024, 16
HG = 8  # heads per core
D = 64  # head dim
OC = HG * D  # 512 output channels per core
N_CORES = 8
MAX_SCALE_MUL = float(np.log(100.0))
WSCALE = 32.0  # host pre-scale on Wq/Wk before fp8 quantization
R = 2048.0  # qhat rescale into fp8 range; exp() applies 1/R

# module-level knobs for test harness
TRACE = False
LAST_RESULT = None

_NC_CACHE = {}


def build_nc():
    f32, bf16, f8 = mybir.dt.float32, mybir.dt.bfloat16, mybir.dt.float8e4
    nc = bacc.Bacc()

    xT8 = nc.declare_dram_parameter("xT8", [C, L], f8, isOutput=False)
    ctxT = nc.declare_dram_parameter("ctxT", [C, LC], bf16, isOutput=False)
    ctxT8 = nc.declare_dram_parameter("ctxT8", [C, LC], f8, isOutput=False)
    wqT = nc.declare_dram_parameter("wqT", [C, OC], f8, isOutput=False)
    wkT = nc.declare_dram_parameter("wkT", [C, OC], f8, isOutput=False)
    wvT = nc.declare_dram_parameter("wvT", [C, OC], bf16, isOutput=False)
    wpT = nc.declare_dram_parameter("wpT", [OC, C], bf16, isOutput=False)
    expbT = nc.declare_dram_parameter("expbT", [HG, LC, L], bf16, isOutput=False)
    hsum = nc.declare_dram_parameter("hsum", [OC, HG], bf16, isOutput=False)
    hbc = nc.declare_dram_parameter("hbc", [HG, OC], bf16, isOutput=False)
    sminv = nc.declare_dram_parameter("sminv", [HG, 1], f32, isOutput=False)
    y = nc.declare_dram_parameter("y", [L, C], bf16, isOutput=True)
    dbg = {}
    if os.environ.get("KDBG", "0") == "1":
        dbg["qhatF"] = nc.declare_dram_parameter("d_qhatF", [128, 2, L], f8, isOutput=True)
        dbg["kTF"] = nc.declare_dram_parameter("d_kTF", [128, 2, LC], f8, isOutput=True)
        dbg["ssb"] = nc.declare_dram_parameter("d_ssb", [HG, L], bf16, isOutput=True)
        dbg["rsk"] = nc.declare_dram_parameter("d_rsk", [128, 4], f32, isOutput=True)
        dbg["qT0"] = nc.declare_dram_parameter("d_qT0", [128, L], bf16, isOutput=True)
        dbg["v0"] = nc.declare_dram_parameter("d_v0", [128, HG, 96], bf16, isOutput=True)
        dbg["praw"] = nc.declare_dram_parameter("d_praw", [128, L], bf16, isOutput=True)
        dbg["ptb"] = nc.declare_dram_parameter("d_ptb", [128, L], bf16, isOutput=True)
        dbg["osb"] = nc.declare_dram_parameter("d_osb", [96, L], f32, isOutput=True)
        dbg["on2"] = nc.declare_dram_parameter("d_on2", [128, L], bf16, isOutput=True)

    KT = C // 128  # 8 contraction tiles (DoubleRow: pairs -> 4 steps)
    OCT = OC // 128  # 4 output-channel tiles
    MT = LC // 128  # 16 context tiles
    LT = L // 128  # 8 query tiles

    with tile.TileContext(nc) as tc, ExitStack() as persist:
        keep = persist.enter_context(tc.tile_pool(name="keep", bufs=1))
        dma = nc.sync

        # head PAIRS stacked across the 128 partitions for the output
        # projection: contraction becomes standard K=128 matmuls
        wp_t = keep.tile([128, HG // 2, C], bf16, tag="wp")
        wp2_sb = [wp_t[:, j, :] for j in range(HG // 2)]

        # fp8 DoubleRow layouts for the S matmul:
        # group g in {0,1} holds heads 4g..4g+3; head h4 = partitions
        # 32*h4..32*h4+31, subtile i covers head-dim 32i..32i+31.
        kTF_sb = [keep.tile([128, 2, LC], f8, tag=f"kTF{g}", name=f"kTF{g}") for g in range(2)]
        qhatF_sb = [keep.tile([128, 2, L], f8, tag=f"qhatF{g}", name=f"qhatF{g}") for g in range(2)]
        # matmul operands must have base partition in {0,32,64}; head 3 of
        # each group lives at base 96, so keep partition-shifted copies
        # (group g at base 32*g of one shared tile).
        kTF96c = keep.tile([64, 2, LC], f8, tag="kTF96c", name="kTF96c")
        qhatF96c = keep.tile([64, 2, L], f8, tag="qhatF96c", name="qhatF96c")
        kTF96_sb = [kTF96c[32 * g : 32 * g + 32, :, :] for g in range(2)]
        qhatF96_sb = [qhatF96c[32 * g : 32 * g + 32, :, :] for g in range(2)]
        v_sb = [keep.tile([128, HG, 96], bf16, tag=f"v{mt}", name=f"v{mt}") for mt in range(MT)]
        on2_sb = [keep.tile([128, L], bf16, tag=f"on2_{j}", name=f"on2_{j}") for j in range(HG // 2)]

        vpool = persist.enter_context(tc.tile_pool(name="vpool", bufs=1))
        ebpool = persist.enter_context(tc.tile_pool(name="ebpool", bufs=4))
        ebt_tiles = {}
        HMT = MT // 4  # mts per ebt quarter-tile

        def load_ebt(h, half):
            t = ebpool.tile([128, HMT, L], bf16, tag="expb", name=f"ebt{h}_{half}")
            lo = half * HMT
            dma.dma_start(
                out=t,
                in_=expbT[h, lo * 128 : (lo + HMT) * 128, :].rearrange(
                    "(g p) l -> p g l", p=128
                ),
            )
            ebt_tiles[(h, half)] = t

        # ---------------- phase 1: projections + norms ----------------
        with ExitStack() as p1:
            wpool = p1.enter_context(tc.tile_pool(name="wpool", bufs=1))
            apool = p1.enter_context(tc.tile_pool(name="apool", bufs=1))
            spool = p1.enter_context(tc.tile_pool(name="spool", bufs=1))
            psA = p1.enter_context(tc.tile_pool(name="psA", bufs=3, space="PSUM"))

            # q-phase inputs first so PE can start ASAP, then k/v inputs
            wq_t = wpool.tile([128, KT, OC], f8, tag="wq")
            wqT_r = wqT.rearrange("(t p) o -> p t o", p=128)
            dma.dma_start(out=wq_t, in_=wqT_r)
            x_t = apool.tile([128, KT, L], f8, tag="x")
            xT_r = xT8.rearrange("(t p) l -> p t l", p=128)
            dma.dma_start(out=x_t, in_=xT_r)
            wk_t = wpool.tile([128, KT, OC], f8, tag="wk")
            dma.dma_start(out=wk_t, in_=wkT.rearrange("(t p) o -> p t o", p=128))
            ctx8_t = apool.tile([128, KT, LC], f8, tag="ctx8")
            dma.dma_start(out=ctx8_t, in_=ctxT8.rearrange("(t p) m -> p t m", p=128))
            hsum_t = wpool.tile([128, OCT, HG], bf16, tag="hsum")
            dma.dma_start(out=hsum_t, in_=hsum.rearrange("(t p) h -> p t h", p=128))
            hsum_sb = [hsum_t[:, ot, :] for ot in range(OCT)]
            hbc_sb = wpool.tile([HG, OC], bf16, tag="hbc")
            dma.dma_start(out=hbc_sb, in_=hbc[:, :])
            sminv_sb = wpool.tile([HG, 1], f32, tag="sminv")
            dma.dma_start(out=sminv_sb, in_=sminv[:, :])
            # head-0 exp(bias) front chunk early so the bias-multiply
            # stream doesn't stall; remainder after ctx/wv
            load_ebt(0, 0)
            load_ebt(0, 2)
            load_ebt(0, 3)
            ctx_t = vpool.tile([128, KT, LC], bf16, tag="ctx")
            dma.dma_start(out=ctx_t, in_=ctxT.rearrange("(t p) m -> p t m", p=128))
            ctx_sb = [ctx_t[:, kt, :] for kt in range(KT)]
            wv_t = vpool.tile([128, KT, OC], bf16, tag="wv")
            dma.dma_start(out=wv_t, in_=wvT.rearrange("(t p) o -> p t o", p=128))
            wv_sb = [wv_t[:, kt, :] for kt in range(KT)]
            load_ebt(0, 1)

            # q projection (fp8 DoubleRow): qT (bf16) and q^2 (bf16)
            qT_sb, q2_sb = [], []
            for ot in range(OCT):
                ps = psA.tile([128, L], f32, tag="psA")
                oc_sl = slice(ot * 128, (ot + 1) * 128)
                for kt in range(0, KT, 2):
                    for nch in range(L // 512):
                        nsl = slice(nch * 512, (nch + 1) * 512)
                        nc.tensor.matmul(
                            ps[:, nsl],
                            wq_t[:, kt : kt + 2, oc_sl],
                            x_t[:, kt : kt + 2, nsl],
                            start=(kt == 0),
                            stop=(kt == KT - 2),
                            perf_mode=PM.DoubleRow,
                        )
                t = apool.tile([128, L], bf16, tag=f"qT{ot}")
                nc.vector.tensor_copy(t, ps)
                qT_sb.append(t)
                t2 = apool.tile([128, L], bf16, tag=f"q2{ot}")
                nc.vector.tensor_mul(t2, t, t)
                q2_sb.append(t2)

            # k projection (fp8 DoubleRow, two Lc halves per oc-tile)
            # + k row norms; evacuate straight to fp8 kTF layout
            n2k_all = spool.tile([128, OCT], f32, tag="n2k_all")
            rsk_all = spool.tile([128, OCT], f32, tag="rsk_all")
            lnk_all = spool.tile([128, OCT], f32, tag="lnk_all")

            def k_proj(ot):
                g, sub = ot // 2, ot % 2
                oc_sl = slice(ot * 128, (ot + 1) * 128)
                n2kh = spool.tile([128, 2], f32, tag=f"n2kh{ot}")
                for half in range(2):
                    ps = psA.tile([128, 1024], f32, tag="psA")
                    for kt in range(0, KT, 2):
                        for nch in range(2):
                            gsl = slice(
                                half * 1024 + nch * 512, half * 1024 + (nch + 1) * 512
                            )
                            nsl = slice(nch * 512, (nch + 1) * 512)
                            nc.tensor.matmul(
                                ps[:, nsl],
                                wk_t[:, kt : kt + 2, oc_sl],
                                ctx8_t[:, kt : kt + 2, gsl],
                                start=(kt == 0),
                                stop=(kt == KT - 2),
                                perf_mode=PM.DoubleRow,
                            )
                    kt_half = kTF_sb[g][:, sub, half * 1024 : (half + 1) * 1024]
                    nc.scalar.activation(kt_half, ps, AF.Copy)
                    k2s = spool.tile([128, 1024], f8, tag="k2s", bufs=2, name="k2s")
                    # k2s = kt*kt with fused row-sum accumulation (from the
                    # fp8 copy; quantization error on the norm is ~0.1%)
                    nc.vector.scalar_tensor_tensor(
                        k2s,
                        kt_half,
                        1.0,
                        kt_half,
                        op0=ALU.mult,
                        op1=ALU.mult,
                        accum_out=n2kh[:, half : half + 1],
                    )
                nc.vector.tensor_add(
                    n2k_all[:, ot : ot + 1], n2kh[:, 0:1], n2kh[:, 1:2]
                )

            # q norms: n2[h,l] -> s = R*sm/sqrt(n2) -> broadcast to rows
            with tc.tile_pool(name="psN", bufs=1, space="PSUM") as psN:
                psn2 = psN.tile([HG, L], f32, tag="psn2")
                for ot in range(OCT):
                    for nch in range(L // 512):
                        nsl = slice(nch * 512, (nch + 1) * 512)
                        nc.tensor.matmul(
                            psn2[:, nsl],
                            hsum_sb[ot],
                            q2_sb[ot][:, nsl],
                            start=(ot == 0),
                            stop=(ot == OCT - 1),
                        )
                k_proj(0)
                k_proj(1)
                k_proj(2)
                k_proj(3)
                nc.scalar.activation(lnk_all, n2k_all, AF.Ln)
                nc.scalar.activation(rsk_all, lnk_all, AF.Exp, scale=-0.5)
                t8 = spool.tile([HG, L], bf16, tag="t8")
                nc.scalar.activation(t8, psn2, AF.Ln, scale=sminv_sb[:, 0:1])
            s_sb = spool.tile([HG, L], bf16, tag="s_sb")
            nc.scalar.activation(s_sb, t8, AF.Exp, scale=-0.5)
            sbc_sb = []
            for ot in range(OCT):
                ps = psA.tile([128, L], f32, tag="psA")
                for nch in range(L // 512):
                    nsl = slice(nch * 512, (nch + 1) * 512)
                    nc.tensor.matmul(
                        ps[:, nsl],
                        hbc_sb[:, ot * 128 : (ot + 1) * 128],
                        s_sb[:, nsl],
                        start=True,
                        stop=True,
                    )
                sbc = spool.tile([128, L], bf16, tag=f"sbc{ot}", name="sbc")
                nc.vector.tensor_copy(sbc, ps)
                sbc_sb.append(sbc)

            # qhat = (qT * rsk_per_partition) * s_broadcast -> fp8 layout
            for ot in range(OCT):
                g, sub = ot // 2, ot % 2
                nc.vector.scalar_tensor_tensor(
                    qhatF_sb[g][:, sub, :],
                    qT_sb[ot],
                    rsk_all[:, ot : ot + 1],
                    sbc_sb[ot],
                    op0=ALU.mult,
                    op1=ALU.mult,
                )
            # partition-shifted copies of head 3 (base 96 -> 0)
            for g in range(2):
                dma.dma_start(out=kTF96_sb[g], in_=kTF_sb[g][96:128, :, :])
                dma.dma_start(out=qhatF96_sb[g], in_=qhatF_sb[g][96:128, :, :])
            if dbg:
                dma.dma_start(out=dbg["qhatF"][:, :, :], in_=qhatF_sb[0])
                dma.dma_start(out=dbg["kTF"][:, :, :], in_=kTF_sb[0])
                dma.dma_start(out=dbg["ssb"][:, :], in_=s_sb)
                dma.dma_start(out=dbg["rsk"][:, :], in_=rsk_all)
                dma.dma_start(out=dbg["qT0"][:, :], in_=qT_sb[0])


        # ---------------- phase 2: attention ----------------
        # Per head: S -> exp -> bias-mult m-tiles stream at the scalar
        # engine's pace (one m-tile per PSUM tile). AV matmuls trail
        # roughly one head behind via a global cursor so exp never waits
        # on PE; the v projection slots in after head 0's stream.
        with ExitStack() as p2:
            stpool = p2.enter_context(tc.tile_pool(name="stream", bufs=4))
            ypool2 = p2.enter_context(tc.tile_pool(name="ypool2", bufs=1))
            yp_box = {"ypart": ypool2.tile([128, LT, C], bf16, tag="ypart", name="ypart")}
            ptbpool = p2.enter_context(tc.tile_pool(name="ptbpool", bufs=17))
            tpool = p2.enter_context(tc.tile_pool(name="tails", bufs=1))
            psS = p2.enter_context(tc.tile_pool(name="psS", bufs=2, space="PSUM"))
            psO = p2.enter_context(tc.tile_pool(name="psO", bufs=1, space="PSUM"))

            ptb_all = {}  # (h, mt) -> ptb tile
            pso_all = {}  # h -> AV psum accumulator
            av_queue = []  # (h, mt), strictly ordered
            emitted_mts = [0] * HG
            v_state = {"done": 0, "psV": None}

            def v_proj_mt(mt):
                # v projection for one m-tile into (m, head, 1+d) layout
                # (ones column first so the AV rowsum lands at partition 0)
                ps = v_state["psV"].tile([128, OC], f32, tag="psV", name="psV")
                msl = slice(mt * 128, (mt + 1) * 128)
                for kt in range(KT):
                    nc.tensor.matmul(
                        ps,
                        ctx_sb[kt][:, msl],
                        wv_sb[kt],
                        start=(kt == 0),
                        stop=(kt == KT - 1),
                    )
                nc.scalar.activation(
                    v_sb[mt][:, :, 32:96],
                    ps.rearrange("p (h d) -> p h d", h=HG),
                    AF.Copy,
                )
                # col 0 = ones (rowsum lands at pso partition 0); cols
                # 1-31 are dead padding so values start at base 32.
                nc.vector.memset(v_sb[mt][:, :, 0:1], 1.0)
                nc.vector.memset(v_sb[mt][:, :, 1:32], 0.0)
                if dbg and mt == 0:
                    dma.dma_start(out=dbg["v0"][:, :, :], in_=v_sb[0])
                v_state["done"] = mt + 1

            tails_done = [0]

            def emit_tail(h):
                tails_done[0] += 1
                pso = pso_all.pop(h)
                # evacuate pso right away so its PSUM banks free for the
                # next head; tail math runs from SBUF. The last head (6)
                # runs the chain in L-halves on DVE so the output
                # projection's first row-blocks can start sooner.
                last = h == 6
                osb = tpool.tile([96, L], f32, tag="osb", bufs=2, name="osb")
                rrec = tpool.tile([1, L], bf16, tag="rrec", bufs=1, name="rrec")
                rb = tpool.tile([96, L], bf16, tag="rb", bufs=1, name="rb")
                if h % 2 == 0:
                    dst = on2_sb[h // 2][0:D, :]
                else:
                    dst = tpool.tile([D, L], bf16, tag="onodd", bufs=1, name="onodd")
                eng = nc.vector if last else nc.gpsimd
                for sl in ([slice(0, 512), slice(512, L)] if last else [slice(0, L)]):
                    nc.vector.tensor_copy(osb[:, sl], pso[:, sl])
                    # rowsum sits at partition 0 (ones column is v[..., 0])
                    with nc.allow_low_precision(reason="bf16 rowsum recip"):
                        nc.vector.reciprocal(rrec[:, sl], osb[0:1, sl])
                    nc.gpsimd.partition_broadcast(rb[:, sl], rrec[:, sl], channels=96)
                    # base-32 APs may touch at most 32 partitions: halves
                    eng.tensor_mul(dst[0:32, sl], osb[32:64, sl], rb[32:64, sl])
                    eng.tensor_mul(dst[32:64, sl], osb[64:96, sl], rb[64:96, sl])
                if h % 2 == 1:
                    dma.dma_start(out=on2_sb[h // 2][D:128, :], in_=dst)
                if dbg and h == 0:
                    dma.dma_start(out=dbg["osb"][:, :], in_=osb)
                if dbg and h == 1:
                    dma.dma_start(out=dbg["on2"][:, :], in_=on2_sb[0])

            def emit_avs(budget):
                while budget > 0 and av_queue:
                    h, mt = av_queue[0]
                    if emitted_mts[h] <= mt or v_state["done"] <= mt:
                        return
                    if h not in pso_all:
                        pso_all[h] = psO.tile([96, L], f32, tag="pso", name="pso")
                    pso = pso_all[h]
                    ptb_tile = ptb_all.pop((h, mt))
                    for nch in range(2):
                        nsl = slice(nch * 512, (nch + 1) * 512)
                        nc.tensor.matmul(
                            pso[:, nsl],
                            v_sb[mt][:, h, :],
                            ptb_tile[:, nsl],
                            start=(mt == 0),
                            stop=(mt == MT - 1),
                        )
                    av_queue.pop(0)
                    budget -= 1
                    if mt == MT - 1:
                        emit_tail(h)

            psYa_box = {"pool": None, "lt": 0}
            v_state["psVcm"] = tc.tile_pool(name="psV", bufs=2, space="PSUM")
            v_state["psV"] = v_state["psVcm"].__enter__()

            def emit_ppass1(lt):
                # first half of the output projection (heads 0..3) into
                # ypart while later heads stream; phase 3 adds j=2,3.
                lsl = slice(lt * 128, (lt + 1) * 128)
                psy = psYa_box["pool"].tile(
                    [128, C], f32, tag="psya", name="psya", bufs=1
                )
                for j in range(2):
                    for nch in range(C // 512):
                        nsl = slice(nch * 512, (nch + 1) * 512)
                        nc.tensor.matmul(
                            psy[:, nsl],
                            on2_sb[j][:, lsl],
                            wp2_sb[j][:, nsl],
                            start=(j == 0),
                            stop=(j == 1),
                        )
                nc.vector.tensor_copy(yp_box["ypart"][:, lt, :], psy)

            HEAD_ORDER = [0, 1, 2, 3, 4, 5, 7, 6]
            for hidx, hh in enumerate(HEAD_ORDER):
                nxt = HEAD_ORDER[hidx + 1] if hidx + 1 < HG else None
                g, h4 = hh // 4, hh % 4
                psl = slice(32 * h4, 32 * h4 + 32)
                if h4 == 3:
                    k_src, q_src = kTF96_sb[g], qhatF96_sb[g]
                else:
                    k_src = kTF_sb[g][psl, :, :]
                    q_src = qhatF_sb[g][psl, :, :]
                if hh == 4:
                    psYa_box["pool"] = p2.enter_context(
                        tc.tile_pool(name="psYa", bufs=1, space="PSUM")
                    )

                if hh == 1:
                    # wp is only read by the output projection
                    dma.dma_start(
                        out=wp_t, in_=wpT.rearrange("(j p) o -> p j o", p=128)
                    )
                for mt in range(MT):
                    # prefetch next head's quarter q-1 only after this
                    # head's quarter q-1 is fully consumed (buffer reuse)
                    if mt % HMT == 0 and mt > 0 and nxt is not None:
                        load_ebt(nxt, mt // HMT - 1)
                    msl = slice(mt * 128, (mt + 1) * 128)
                    pss = psS.tile([128, L], f32, tag="pss", name="pss")
                    for nch in range(2):
                        nsl = slice(nch * 512, (nch + 1) * 512)
                        nc.tensor.matmul(
                            pss[:, nsl],
                            k_src[:, :, msl],
                            q_src[:, :, nsl],
                            start=True,
                            stop=True,
                            perf_mode=PM.DoubleRow,
                        )
                    praw = stpool.tile([128, L], bf16, tag="praw", name="praw")
                    nc.scalar.activation(praw, pss, AF.Exp, scale=1.0 / R)
                    n_live = len(ptb_all) + 1
                    assert n_live <= 17, f"ptb live {n_live} exceeds pool bufs"
                    ptb = ptbpool.tile([128, L], bf16, tag="ptb", name="ptb")
                    nc.vector.tensor_mul(
                        ptb, praw, ebt_tiles[(hh, mt // HMT)][:, mt % HMT, :]
                    )
                    if mt % HMT == HMT - 1:
                        ebt_tiles.pop((hh, mt // HMT))
                    if dbg and hh == 0 and mt == 0:
                        dma.dma_start(out=dbg["praw"][:, :], in_=praw)
                        dma.dma_start(out=dbg["ptb"][:, :], in_=ptb)
                    ptb_all[(hh, mt)] = ptb
                    emitted_mts[hh] = mt + 1
                    av_queue.append((hh, mt))
                    if hh == 0 and mt >= 6:
                        v_proj_mt(mt - 6)
                    if hh == 1 and 10 + mt // 2 < MT and mt % 2 == 0:
                        v_proj_mt(10 + mt // 2)
                    if (
                        psYa_box["lt"] < LT
                        and psYa_box["pool"] is not None
                        and tails_done[0] >= 4
                        and (mt % 8 == 6)
                    ):
                        emit_ppass1(psYa_box["lt"])
                        psYa_box["lt"] += 1
                    emit_avs(3 if len(av_queue) > 12 else (2 if len(av_queue) > 6 or hidx == HG - 1 else 1))

                if nxt is not None:
                    load_ebt(nxt, 3)
                if hh == 1:
                    v_state["psVcm"].__exit__(None, None, None)

            while av_queue:
                emit_avs(1000)
            while psYa_box["lt"] < LT:
                emit_ppass1(psYa_box["lt"])
                psYa_box["lt"] += 1

            # ---------------- phase 3: output projection ----------------
            ypool = p2.enter_context(tc.tile_pool(name="ypool", bufs=2))

            for lt in range(LT):
                lsl = slice(lt * 128, (lt + 1) * 128)
                ysb = ypool.tile([128, C], bf16, tag="ysb")
                psy = psS.tile([128, L], f32, tag="pss", name="pss")
                for j in range(2, HG // 2):
                    for nch in range(C // 512):
                        nsl = slice(nch * 512, (nch + 1) * 512)
                        nc.tensor.matmul(
                            psy[:, nsl],
                            on2_sb[j][:, lsl],
                            wp2_sb[j][:, nsl],
                            start=(j == 2),
                            stop=(j == HG // 2 - 1),
                        )
                nc.vector.tensor_add(ysb, psy, yp_box["ypart"][:, lt, :])
                dma.dma_start(out=y[lsl, :], in_=ysb)

    nc.compile()
    return nc


def _get_nc():
    if "nc" not in _NC_CACHE:
        _NC_CACHE["nc"] = build_nc()
    return _NC_CACHE["nc"]


def _col_perm():
    """New column order within a core's OC block: tile t in 0..3 holds
    heads 4*(t//2)..4*(t//2)+3, head-dim lanes 32*(t%2)..32*(t%2)+31;
    partition p maps to (h = 4*(t//2) + p//32, d = 32*(t%2) + p%32).
    Returns perm with perm[t*128 + p] = old column index h*64 + d."""
    perm = np.empty(OC, dtype=np.int64)
    for t in range(4):
        for p in range(128):
            h = 4 * (t // 2) + p // 32
            d = 32 * (t % 2) + p % 32
            perm[t * 128 + p] = h * 64 + d
    return perm


def kernel(x, context, attn_bias, Wq, Wk, Wv, Wp, bp, scale_mul):
    global LAST_RESULT
    x = np.asarray(x, dtype=np.float32)
    context = np.asarray(context, dtype=np.float32)
    attn_bias = np.asarray(attn_bias, dtype=np.float32)
    Wq = np.asarray(Wq, dtype=np.float32)
    Wk = np.asarray(Wk, dtype=np.float32)
    Wv = np.asarray(Wv, dtype=np.float32)
    Wp = np.asarray(Wp, dtype=np.float32)
    bp = np.asarray(bp, dtype=np.float32)
    scale_mul = np.asarray(scale_mul, dtype=np.float32)

    sm = np.exp(np.minimum(scale_mul, MAX_SCALE_MUL)).reshape(H)  # (H,)
    expb = np.exp(attn_bias[0])  # (H, L, Lc)

    perm = _col_perm()
    # hsum/hbc map partitions of tile t to heads under the new order
    hsum = np.zeros((OC, HG), dtype=BF16)
    hbc = np.zeros((HG, OC), dtype=BF16)
    for t in range(4):
        for p in range(128):
            h = 4 * (t // 2) + p // 32
            hsum[t * 128 + p, h] = 1.0
            hbc[h, t * 128 + p] = 1.0

    gshard = {}
    for g in range(2):
        rows = slice(g * OC, (g + 1) * OC)
        heads = slice(g * HG, (g + 1) * HG)
        wq_g = (WSCALE * Wq[rows, :])[perm, :]  # [OC, C] reordered rows
        wk_g = (WSCALE * Wk[rows, :])[perm, :]
        gshard[g] = dict(
            wqT=np.ascontiguousarray(wq_g.T).astype(F8),
            wkT=np.ascontiguousarray(wk_g.T).astype(F8),
            wvT=np.ascontiguousarray(Wv[rows, :].T).astype(BF16),
            wpT=np.ascontiguousarray(Wp[:, rows].T).astype(BF16),
            expbT=np.ascontiguousarray(
                np.transpose(expb[heads], (0, 2, 1))
            ).astype(BF16),
            # s = (psn2 * sminv)^-0.5 should equal R*sm/sqrt(psn2)
            sminv=(1.0 / (R * R * sm[heads] ** 2)).reshape(HG, 1).astype(np.float32),
        )
    bshard = {}
    for b in range(B):
        xb = np.ascontiguousarray(x[b].T)
        cb = np.ascontiguousarray(context[b].T)
        bshard[b] = dict(
            xT8=xb.astype(F8),
            ctxT=cb.astype(BF16),
            ctxT8=cb.astype(F8),
        )

    in_maps = []
    for core in range(N_CORES):
        b, g = core // 2, core % 2
        m = dict(hsum=hsum, hbc=hbc)
        m.update(gshard[g])
        m.update(bshard[b])
        in_maps.append(m)

    nc = _get_nc()
    res = run_bass_kernel_spmd(
        nc, in_maps, core_ids=list(range(N_CORES)), trace=TRACE
    )
    LAST_RESULT = res
    outs = [r["y"].astype(np.float32) for r in res.results]
    out = np.stack(
        [outs[2 * b] + outs[2 * b + 1] + bp[None, :] for b in range(B)]
    ).astype(np.float32)
    return out


# revision 87
# speedup vs baseline: 1.0502x; 1.0384x over previous
"""Distributed Trainium2 Bass kernel for nn_CrossAttention (B=4, L=1024,
Lc=2048, C=1024, H=16).

Sharding: 8 cores = 4 batches x 2 head-groups of 8 heads. Each core
computes its batch's q/k/v projections for its 8 heads, the attention,
and a partial output projection (row-shard of Wp). Host sums the two
partial outputs per batch (bf16) and adds bp.

Precision: q/k projections and the S=q@k^T matmul run in fp8e4 with
perf_mode=DoubleRow (two contraction subtiles per instruction -> 2x PE
throughput, K=256 per instruction for the projections). Wq/Wk are
pre-scaled by 32 on the host (q/k are l2-normalized downstream so the
scale cancels); qhat is rescaled by R=2048 into fp8 range and exp()
compensates with scale=1/R. Weight columns are reordered host-side so
each 128-partition tile holds 4 heads x 32 head-dim lanes, giving the
S matmul its [32, 2, *] DoubleRow layout directly. V/AV/P-projection
stay bf16 for accuracy.

Schedule: per head, S -> exp -> bias-multiply m-tiles stream at the
scalar engine's pace (exp is the global bottleneck at ~134us busy).
AV matmuls trail roughly one head behind via a global cursor so exp
never stalls on PE; the v projection interleaves into heads 0-1; the
first half of the output projection (head pairs j=0,1) runs in-stream
into an SBUF partial (ypart) once heads 0-3 finish; heads run in order
[0,1,2,3,4,5,7,6] so the final tail avoids the cross-partition DMA.
The softmax denominator rides as a leading ones-column in V (rowsum
lands at PSUM partition 0 for partition_broadcast); values sit at
partitions 32..95 to satisfy base-partition rules.
"""

import os
import sys
from contextlib import ExitStack

sys.path.insert(0, "/opt/trn_rl_repo")

import numpy as np
import ml_dtypes

import concourse.bass as bass
from concourse import bacc
import concourse.mybir as mybir
import concourse.tile as tile
from concourse.bass_utils import run_bass_kernel_spmd

BF16 = ml_dtypes.bfloat16
F8 = ml_dtypes.float8_e4m3
AF = mybir.ActivationFunctionType
ALU = mybir.AluOpType
AX = mybir.AxisListType
PM = mybir.MatmulPerfMode

# All ACT functions used here (Copy/Exp/Ln) live in the
# natural_log_exp_and_others table set; blank the other sets so
# insert_act_table_loads emits exactly one table load.
from concourse.hw_specs import get_activation_tables as _gat_orig


def _gat_one_set(arch):
    t = _gat_orig(arch)
    return {
        k: (v if k == "natural_log_exp_and_others" else set()) for k, v in t.items()
    }


bacc.get_activation_tables = _gat_one_set

B, L, LC, C, H = 4, 1024, 2048, 1024, 16
HG = 8  # heads per core
D = 64  # head dim
OC = HG * D  # 512 output channels per core
N_CORES = 8
MAX_SCALE_MUL = float(np.log(100.0))
WSCALE = 32.0  # host pre-scale on Wq/Wk before fp8 quantization
R = 2048.0  # qhat rescale into fp8 range; exp() applies 1/R

# module-level knobs for test harness
TRACE = False
LAST_RESULT = None

_NC_CACHE = {}


def build_nc():
    f32, bf16, f8 = mybir.dt.float32, mybir.dt.bfloat16, mybir.dt.float8e4
    nc = bacc.Bacc()

    xT8 = nc.declare_dram_parameter("xT8", [C, L], f8, isOutput=False)
    ctxT = nc.declare_dram_parameter("ctxT", [C, LC], bf16, isOutput=False)
    ctxT8 = nc.declare_dram_parameter("ctxT8", [C, LC], f8, isOutput=False)
    wqT = nc.declare_dram_parameter("wqT", [C, OC], f8, isOutput=False)
    wkT = nc.declare_dram_parameter("wkT", [C, OC], f8, isOutput=False)
    wvT = nc.declare_dram_parameter("wvT", [C, OC], bf16, isOutput=False)
    wpT = nc.declare_dram_parameter("wpT", [OC, C], bf16, isOutput=False)
    expbT = nc.declare_dram_parameter("expbT", [HG, LC, L], bf16, isOutput=False)
    hsum = nc.declare_dram_parameter("hsum", [OC, HG], bf16, isOutput=False)
    hbc = nc.declare_dram_parameter("hbc", [HG, OC], bf16, isOutput=False)
    sminv = nc.declare_dram_parameter("sminv", [HG, 1], f32, isOutput=False)
    y = nc.declare_dram_parameter("y", [L, C], bf16, isOutput=True)
    dbg = {}
    if os.environ.get("KDBG", "0") == "1":
        dbg["qhatF"] = nc.declare_dram_parameter("d_qhatF", [128, 2, L], f8, isOutput=True)
        dbg["kTF"] = nc.declare_dram_parameter("d_kTF", [128, 2, LC], f8, isOutput=True)
        dbg["ssb"] = nc.declare_dram_parameter("d_ssb", [HG, L], bf16, isOutput=True)
        dbg["rsk"] = nc.declare_dram_parameter("d_rsk", [128, 4], f32, isOutput=True)
        dbg["qT0"] = nc.declare_dram_parameter("d_qT0", [128, L], bf16, isOutput=True)
        dbg["v0"] = nc.declare_dram_parameter("d_v0", [128, HG, 96], bf16, isOutput=True)
        dbg["praw"] = nc.declare_dram_parameter("d_praw", [128, L], bf16, isOutput=True)
        dbg["ptb"] = nc.declare_dram_parameter("d_ptb", [128, L], bf16, isOutput=True)
        dbg["osb"] = nc.declare_dram_parameter("d_osb", [96, L], f32, isOutput=True)
        dbg["on2"] = nc.declare_dram_parameter("d_on2", [128, L], bf16, isOutput=True)

    KT = C // 128  # 8 contraction tiles (DoubleRow: pairs -> 4 steps)
    OCT = OC // 128  # 4 output-channel tiles
    MT = LC // 128  # 16 context tiles
    LT = L // 128  # 8 query tiles

    with tile.TileContext(nc) as tc, ExitStack() as persist:
        keep = persist.enter_context(tc.tile_pool(name="keep", bufs=1))
        dma = nc.sync

        # head PAIRS stacked across the 128 partitions for the output
        # projection: contraction becomes standard K=128 matmuls
        wp_t = keep.tile([128, HG // 2, C], bf16, tag="wp")
        wp2_sb = [wp_t[:, j, :] for j in range(HG // 2)]

        # fp8 DoubleRow layouts for the S matmul:
        # group g in {0,1} holds heads 4g..4g+3; head h4 = partitions
        # 32*h4..32*h4+31, subtile i covers head-dim 32i..32i+31.
        kTF_sb = [keep.tile([128, 2, LC], f8, tag=f"kTF{g}", name=f"kTF{g}") for g in range(2)]
        qhatF_sb = [keep.tile([128, 2, L], f8, tag=f"qhatF{g}", name=f"qhatF{g}") for g in range(2)]
        # matmul operands must have base partition in {0,32,64}; head 3 of
        # each group lives at base 96, so keep partition-shifted copies
        # (group g at base 32*g of one shared tile).
        kTF96c = keep.tile([64, 2, LC], f8, tag="kTF96c", name="kTF96c")
        qhatF96c = keep.tile([64, 2, L], f8, tag="qhatF96c", name="qhatF96c")
        kTF96_sb = [kTF96c[32 * g : 32 * g + 32, :, :] for g in range(2)]
        qhatF96_sb = [qhatF96c[32 * g : 32 * g + 32, :, :] for g in range(2)]
        v_sb = [keep.tile([128, HG, 96], bf16, tag=f"v{mt}", name=f"v{mt}") for mt in range(MT)]
        on2_sb = [keep.tile([128, L], bf16, tag=f"on2_{j}", name=f"on2_{j}") for j in range(HG // 2)]

        vpool = persist.enter_context(tc.tile_pool(name="vpool", bufs=1))
        ebpool = persist.enter_context(tc.tile_pool(name="ebpool", bufs=4))
        ebt_tiles = {}
        HMT = MT // 4  # mts per ebt quarter-tile

        def load_ebt(h, half):
            t = ebpool.tile([128, HMT, L], bf16, tag="expb", name=f"ebt{h}_{half}")
            lo = half * HMT
            dma.dma_start(
                out=t,
                in_=expbT[h, lo * 128 : (lo + HMT) * 128, :].rearrange(
                    "(g p) l -> p g l", p=128
                ),
            )
            ebt_tiles[(h, half)] = t

        # ---------------- phase 1: projections + norms ----------------
        with ExitStack() as p1:
            wpool = p1.enter_context(tc.tile_pool(name="wpool", bufs=1))
            apool = p1.enter_context(tc.tile_pool(name="apool", bufs=1))
            spool = p1.enter_context(tc.tile_pool(name="spool", bufs=1))
            psA = p1.enter_context(tc.tile_pool(name="psA", bufs=3, space="PSUM"))

            # q-phase inputs first so PE can start ASAP, then k/v inputs
            wq_t = wpool.tile([128, KT, OC], f8, tag="wq")
            wqT_r = wqT.rearrange("(t p) o -> p t o", p=128)
            dma.dma_start(out=wq_t, in_=wqT_r)
            x_t = apool.tile([128, KT, L], f8, tag="x")
            xT_r = xT8.rearrange("(t p) l -> p t l", p=128)
            dma.dma_start(out=x_t, in_=xT_r)
            wk_t = wpool.tile([128, KT, OC], f8, tag="wk")
            dma.dma_start(out=wk_t, in_=wkT.rearrange("(t p) o -> p t o", p=128))
            ctx8_t = apool.tile([128, KT, LC], f8, tag="ctx8")
            dma.dma_start(out=ctx8_t, in_=ctxT8.rearrange("(t p) m -> p t m", p=128))
            hsum_t = wpool.tile([128, OCT, HG], bf16, tag="hsum")
            dma.dma_start(out=hsum_t, in_=hsum.rearrange("(t p) h -> p t h", p=128))
            hsum_sb = [hsum_t[:, ot, :] for ot in range(OCT)]
            hbc_sb = wpool.tile([HG, OC], bf16, tag="hbc")
            dma.dma_start(out=hbc_sb, in_=hbc[:, :])
            sminv_sb = wpool.tile([HG, 1], f32, tag="sminv")
            dma.dma_start(out=sminv_sb, in_=sminv[:, :])
            # head-0 exp(bias) front chunk early so the bias-multiply
            # stream doesn't stall; remainder after ctx/wv
            load_ebt(0, 0)
            load_ebt(0, 2)
            load_ebt(0, 3)
            ctx_t = vpool.tile([128, KT, LC], bf16, tag="ctx")
            dma.dma_start(out=ctx_t, in_=ctxT.rearrange("(t p) m -> p t m", p=128))
            ctx_sb = [ctx_t[:, kt, :] for kt in range(KT)]
            wv_t = vpool.tile([128, KT, OC], bf16, tag="wv")
            dma.dma_start(out=wv_t, in_=wvT.rearrange("(t p) o -> p t o", p=128))
            wv_sb = [wv_t[:, kt, :] for kt in range(KT)]
            load_ebt(0, 1)

            # q projection (fp8 DoubleRow): qT (bf16) and q^2 (bf16)
            qT_sb, q2_sb = [], []
            for ot in range(OCT):
                ps = psA.tile([128, L], f32, tag="psA")
                oc_sl = slice(ot * 128, (ot + 1) * 128)
                for kt in range(0, KT, 2):
                    for nch in range(L // 512):
                        nsl = slice(nch * 512, (nch + 1) * 512)
                        nc.tensor.matmul(
                            ps[:, nsl],
                            wq_t[:, kt : kt + 2, oc_sl],
                            x_t[:, kt : kt + 2, nsl],
                            start=(kt == 0),
                            stop=(kt == KT - 2),
                            perf_mode=PM.DoubleRow,
                        )
                t = apool.tile([128, L], bf16, tag=f"qT{ot}")
                nc.vector.tensor_copy(t, ps)
                qT_sb.append(t)
                t2 = apool.tile([128, L], bf16, tag=f"q2{ot}")
                nc.vector.tensor_mul(t2, t, t)
                q2_sb.append(t2)

            # k projection (fp8 DoubleRow, two Lc halves per oc-tile)
            # + k row norms; evacuate straight to fp8 kTF layout
            n2k_all = spool.tile([128, OCT], f32, tag="n2k_all")
            rsk_all = spool.tile([128, OCT], f32, tag="rsk_all")
            lnk_all = spool.tile([128, OCT], f32, tag="lnk_all")

            def k_proj(ot):
                g, sub = ot // 2, ot % 2
                oc_sl = slice(ot * 128, (ot + 1) * 128)
                n2kh = spool.tile([128, 2], f32, tag=f"n2kh{ot}")
                for half in range(2):
                    ps = psA.tile([128, 1024], f32, tag="psA")
                    for kt in range(0, KT, 2):
                        for nch in range(2):
                            gsl = slice(
                                half * 1024 + nch * 512, half * 1024 + (nch + 1) * 512
                            )
                            nsl = slice(nch * 512, (nch + 1) * 512)
                            nc.tensor.matmul(
                                ps[:, nsl],
                                wk_t[:, kt : kt + 2, oc_sl],
                                ctx8_t[:, kt : kt + 2, gsl],
                                start=(kt == 0),
                                stop=(kt == KT - 2),
                                perf_mode=PM.DoubleRow,
                            )
                    kt_half = kTF_sb[g][:, sub, half * 1024 : (half + 1) * 1024]
                    nc.scalar.activation(kt_half, ps, AF.Copy)
                    k2s = spool.tile([128, 1024], f8, tag="k2s", bufs=2, name="k2s")
                    # k2s = kt*kt with fused row-sum accumulation (from the
                    # fp8 copy; quantization error on the norm is ~0.1%)
                    nc.vector.scalar_tensor_tensor(
                        k2s,
                        kt_half,
                        1.0,
                        kt_half,
                        op0=ALU.mult,
                        op1=ALU.mult,
                        accum_out=n2kh[:, half : half + 1],
                    )
                nc.vector.tensor_add(
                    n2k_all[:, ot : ot + 1], n2kh[:, 0:1], n2kh[:, 1:2]
                )

            # q norms: n2[h,l] -> s = R*sm/sqrt(n2) -> broadcast to rows
            with tc.tile_pool(name="psN", bufs=1, space="PSUM") as psN:
                psn2 = psN.tile([HG, L], f32, tag="psn2")
                for ot in range(OCT):
                    for nch in range(L // 512):
                        nsl = slice(nch * 512, (nch + 1) * 512)
                        nc.tensor.matmul(
                            psn2[:, nsl],
                            hsum_sb[ot],
                            q2_sb[ot][:, nsl],
                            start=(ot == 0),
                            stop=(ot == OCT - 1),
                        )
                k_proj(0)
                k_proj(1)
                k_proj(2)
                k_proj(3)
                nc.scalar.activation(lnk_all, n2k_all, AF.Ln)
                nc.scalar.activation(rsk_all, lnk_all, AF.Exp, scale=-0.5)
                t8 = spool.tile([HG, L], bf16, tag="t8")
                nc.scalar.activation(t8, psn2, AF.Ln, scale=sminv_sb[:, 0:1])
            s_sb = spool.tile([HG, L], bf16, tag="s_sb")
            nc.scalar.activation(s_sb, t8, AF.Exp, scale=-0.5)
            sbc_sb = []
            for ot in range(OCT):
                ps = psA.tile([128, L], f32, tag="psA")
                for nch in range(L // 512):
                    nsl = slice(nch * 512, (nch + 1) * 512)
                    nc.tensor.matmul(
                        ps[:, nsl],
                        hbc_sb[:, ot * 128 : (ot + 1) * 128],
                        s_sb[:, nsl],
                        start=True,
                        stop=True,
                    )
                sbc = spool.tile([128, L], bf16, tag=f"sbc{ot}", name="sbc")
                nc.vector.tensor_copy(sbc, ps)
                sbc_sb.append(sbc)

            # qhat = (qT * rsk_per_partition) * s_broadcast -> fp8 layout
            for ot in range(OCT):
                g, sub = ot // 2, ot % 2
                nc.vector.scalar_tensor_tensor(
                    qhatF_sb[g][:, sub, :],
                    qT_sb[ot],
                    rsk_all[:, ot : ot + 1],
                    sbc_sb[ot],
                    op0=ALU.mult,
                    op1=ALU.mult,
                )
            # partition-shifted copies of head 3 (base 96 -> 0)
            for g in range(2):
                dma.dma_start(out=kTF96_sb[g], in_=kTF_sb[g][96:128, :, :])
                dma.dma_start(out=qhatF96_sb[g], in_=qhatF_sb[g][96:128, :, :])
            if dbg:
                dma.dma_start(out=dbg["qhatF"][:, :, :], in_=qhatF_sb[0])
                dma.dma_start(out=dbg["kTF"][:, :, :], in_=kTF_sb[0])
                dma.dma_start(out=dbg["ssb"][:, :], in_=s_sb)
                dma.dma_start(out=dbg["rsk"][:, :], in_=rsk_all)
                dma.dma_start(out=dbg["qT0"][:, :], in_=qT_sb[0])


        # ---------------- phase 2: attention ----------------
        # Per head: S -> exp -> bias-mult m-tiles stream at the scalar
        # engine's pace (one m-tile per PSUM tile). AV matmuls trail
        # roughly one head behind via a global cursor so exp never waits
        # on PE; the v projection slots in after head 0's stream.
        with ExitStack() as p2:
            stpool = p2.enter_context(tc.tile_pool(name="stream", bufs=4))
            ypool2 = p2.enter_context(tc.tile_pool(name="ypool2", bufs=1))
            yp_box = {"ypart": ypool2.tile([128, LT, C], bf16, tag="ypart", name="ypart")}
            ptbpool = p2.enter_context(tc.tile_pool(name="ptbpool", bufs=17))
            tpool = p2.enter_context(tc.tile_pool(name="tails", bufs=1))
            psS = p2.enter_context(tc.tile_pool(name="psS", bufs=2, space="PSUM"))
            psO = p2.enter_context(tc.tile_pool(name="psO", bufs=1, space="PSUM"))

            ptb_all = {}  # (h, mt) -> ptb tile
            pso_all = {}  # h -> AV psum accumulator
            av_queue = []  # (h, mt), strictly ordered
            emitted_mts = [0] * HG
            v_state = {"done": 0, "psV": None}

            def v_proj_mt(mt):
                # v projection for one m-tile into (m, head, 1+d) layout
                # (ones column first so the AV rowsum lands at partition 0)
                ps = v_state["psV"].tile([128, OC], f32, tag="psV", name="psV")
                msl = slice(mt * 128, (mt + 1) * 128)
                for kt in range(KT):
                    nc.tensor.matmul(
                        ps,
                        ctx_sb[kt][:, msl],
                        wv_sb[kt],
                        start=(kt == 0),
                        stop=(kt == KT - 1),
                    )
                nc.scalar.activation(
                    v_sb[mt][:, :, 32:96],
                    ps.rearrange("p (h d) -> p h d", h=HG),
                    AF.Copy,
                )
                # col 0 = ones (rowsum lands at pso partition 0); cols
                # 1-31 are dead padding so values start at base 32.
                nc.vector.memset(v_sb[mt][:, :, 0:1], 1.0)
                nc.vector.memset(v_sb[mt][:, :, 1:32], 0.0)
                if dbg and mt == 0:
                    dma.dma_start(out=dbg["v0"][:, :, :], in_=v_sb[0])
                v_state["done"] = mt + 1

            tails_done = [0]

            def emit_tail(h):
                tails_done[0] += 1
                pso = pso_all.pop(h)
                # evacuate pso right away so its PSUM banks free for the
                # next head; tail math runs from SBUF. The last head (6)
                # runs the chain in L-halves on DVE so the output
                # projection's first row-blocks can start sooner.
                last = h == 6
                osb = tpool.tile([96, L], f32, tag="osb", bufs=2, name="osb")
                rrec = tpool.tile([1, L], bf16, tag="rrec", bufs=1, name="rrec")
                rb = tpool.tile([96, L], bf16, tag="rb", bufs=1, name="rb")
                if h % 2 == 0:
                    dst = on2_sb[h // 2][0:D, :]
                else:
                    dst = tpool.tile([D, L], bf16, tag="onodd", bufs=1, name="onodd")
                eng = nc.vector if last else nc.gpsimd
                for sl in ([slice(0, 256), slice(256, 512), slice(512, 768), slice(768, L)] if last else [slice(0, L)]):
                    nc.vector.tensor_copy(osb[:, sl], pso[:, sl])
                    # rowsum sits at partition 0 (ones column is v[..., 0])
                    with nc.allow_low_precision(reason="bf16 rowsum recip"):
                        nc.vector.reciprocal(rrec[:, sl], osb[0:1, sl])
                    nc.gpsimd.partition_broadcast(rb[:, sl], rrec[:, sl], channels=96)
                    # base-32 APs may touch at most 32 partitions: halves
                    eng.tensor_mul(dst[0:32, sl], osb[32:64, sl], rb[32:64, sl])
                    eng.tensor_mul(dst[32:64, sl], osb[64:96, sl], rb[64:96, sl])
                if h % 2 == 1:
                    dma.dma_start(out=on2_sb[h // 2][D:128, :], in_=dst)
                if dbg and h == 0:
                    dma.dma_start(out=dbg["osb"][:, :], in_=osb)
                if dbg and h == 1:
                    dma.dma_start(out=dbg["on2"][:, :], in_=on2_sb[0])

            def emit_avs(budget):
                while budget > 0 and av_queue:
                    h, mt = av_queue[0]
                    if emitted_mts[h] <= mt or v_state["done"] <= mt:
                        return
                    if h not in pso_all:
                        pso_all[h] = psO.tile([96, L], f32, tag="pso", name="pso")
                    pso = pso_all[h]
                    ptb_tile = ptb_all.pop((h, mt))
                    for nch in range(2):
                        nsl = slice(nch * 512, (nch + 1) * 512)
                        nc.tensor.matmul(
                            pso[:, nsl],
                            v_sb[mt][:, h, :],
                            ptb_tile[:, nsl],
                            start=(mt == 0),
                            stop=(mt == MT - 1),
                        )
                    av_queue.pop(0)
                    budget -= 1
                    if mt == MT - 1:
                        emit_tail(h)

            psYa_box = {"pool": None, "lt": 0}
            v_state["psVcm"] = tc.tile_pool(name="psV", bufs=2, space="PSUM")
            v_state["psV"] = v_state["psVcm"].__enter__()

            def emit_ppass1(lt):
                # first half of the output projection (heads 0..3) into
                # ypart while later heads stream; phase 3 adds j=2,3.
                lsl = slice(lt * 128, (lt + 1) * 128)
                psy = psYa_box["pool"].tile(
                    [128, C], f32, tag="psya", name="psya", bufs=1
                )
                for j in range(2):
                    for nch in range(C // 512):
                        nsl = slice(nch * 512, (nch + 1) * 512)
                        nc.tensor.matmul(
                            psy[:, nsl],
                            on2_sb[j][:, lsl],
                            wp2_sb[j][:, nsl],
                            start=(j == 0),
                            stop=(j == 1),
                        )
                nc.vector.tensor_copy(yp_box["ypart"][:, lt, :], psy)

            HEAD_ORDER = [0, 1, 2, 3, 4, 5, 7, 6]
            for hidx, hh in enumerate(HEAD_ORDER):
                nxt = HEAD_ORDER[hidx + 1] if hidx + 1 < HG else None
                g, h4 = hh // 4, hh % 4
                psl = slice(32 * h4, 32 * h4 + 32)
                if h4 == 3:
                    k_src, q_src = kTF96_sb[g], qhatF96_sb[g]
                else:
                    k_src = kTF_sb[g][psl, :, :]
                    q_src = qhatF_sb[g][psl, :, :]
                if hh == 4:
                    psYa_box["pool"] = p2.enter_context(
                        tc.tile_pool(name="psYa", bufs=1, space="PSUM")
                    )

                if hh == 1:
                    # wp is only read by the output projection
                    dma.dma_start(
                        out=wp_t, in_=wpT.rearrange("(j p) o -> p j o", p=128)
                    )
                for mt in range(MT):
                    # prefetch next head's quarter q-1 only after this
                    # head's quarter q-1 is fully consumed (buffer reuse)
                    if mt % HMT == 0 and mt > 0 and nxt is not None:
                        load_ebt(nxt, mt // HMT - 1)
                    msl = slice(mt * 128, (mt + 1) * 128)
                    pss = psS.tile([128, L], f32, tag="pss", name="pss")
                    with tc.high_priority(offset=100):
                        for nch in range(2):
                            nsl = slice(nch * 512, (nch + 1) * 512)
                            nc.tensor.matmul(
                                pss[:, nsl],
                                k_src[:, :, msl],
                                q_src[:, :, nsl],
                                start=True,
                                stop=True,
                                perf_mode=PM.DoubleRow,
                            )
                    praw = stpool.tile([128, L], bf16, tag="praw", name="praw")
                    nc.scalar.activation(praw, pss, AF.Exp, scale=1.0 / R)
                    n_live = len(ptb_all) + 1
                    assert n_live <= 17, f"ptb live {n_live} exceeds pool bufs"
                    ptb = ptbpool.tile([128, L], bf16, tag="ptb", name="ptb")
                    nc.vector.tensor_mul(
                        ptb, praw, ebt_tiles[(hh, mt // HMT)][:, mt % HMT, :]
                    )
                    if mt % HMT == HMT - 1:
                        ebt_tiles.pop((hh, mt // HMT))
                    if dbg and hh == 0 and mt == 0:
                        dma.dma_start(out=dbg["praw"][:, :], in_=praw)
                        dma.dma_start(out=dbg["ptb"][:, :], in_=ptb)
                    ptb_all[(hh, mt)] = ptb
                    emitted_mts[hh] = mt + 1
                    av_queue.append((hh, mt))
                    if hh == 0 and mt >= 6:
                        v_proj_mt(mt - 6)
                    if hh == 1 and 10 + mt // 2 < MT and mt % 2 == 0:
                        v_proj_mt(10 + mt // 2)
                    if (
                        psYa_box["lt"] < LT
                        and psYa_box["pool"] is not None
                        and tails_done[0] >= 4
                        and (mt % 8 == 6)
                    ):
                        emit_ppass1(psYa_box["lt"])
                        psYa_box["lt"] += 1
                    emit_avs(3 if len(av_queue) > 12 else (2 if len(av_queue) > 6 or hidx == HG - 1 else 1))

                if nxt is not None:
                    load_ebt(nxt, 3)
                if hh == 1:
                    v_state["psVcm"].__exit__(None, None, None)

            while av_queue:
                emit_avs(1000)
            while psYa_box["lt"] < LT:
                emit_ppass1(psYa_box["lt"])
                psYa_box["lt"] += 1

            # ---------------- phase 3: output projection ----------------
            ypool = p2.enter_context(tc.tile_pool(name="ypool", bufs=2))

            for lt in range(LT):
                lsl = slice(lt * 128, (lt + 1) * 128)
                ysb = ypool.tile([128, C], bf16, tag="ysb")
                psy = psS.tile([128, L], f32, tag="pss", name="pss")
                for j in range(2, HG // 2):
                    for nch in range(C // 512):
                        nsl = slice(nch * 512, (nch + 1) * 512)
                        nc.tensor.matmul(
                            psy[:, nsl],
                            on2_sb[j][:, lsl],
                            wp2_sb[j][:, nsl],
                            start=(j == 2),
                            stop=(j == HG // 2 - 1),
                        )
                nc.vector.tensor_add(ysb, psy, yp_box["ypart"][:, lt, :])
                dma.dma_start(out=y[lsl, :], in_=ysb)

    nc.compile()
    return nc


def _get_nc():
    if "nc" not in _NC_CACHE:
        _NC_CACHE["nc"] = build_nc()
    return _NC_CACHE["nc"]


def _col_perm():
    """New column order within a core's OC block: tile t in 0..3 holds
    heads 4*(t//2)..4*(t//2)+3, head-dim lanes 32*(t%2)..32*(t%2)+31;
    partition p maps to (h = 4*(t//2) + p//32, d = 32*(t%2) + p%32).
    Returns perm with perm[t*128 + p] = old column index h*64 + d."""
    perm = np.empty(OC, dtype=np.int64)
    for t in range(4):
        for p in range(128):
            h = 4 * (t // 2) + p // 32
            d = 32 * (t % 2) + p % 32
            perm[t * 128 + p] = h * 64 + d
    return perm


def kernel(x, context, attn_bias, Wq, Wk, Wv, Wp, bp, scale_mul):
    global LAST_RESULT
    x = np.asarray(x, dtype=np.float32)
    context = np.asarray(context, dtype=np.float32)
    attn_bias = np.asarray(attn_bias, dtype=np.float32)
    Wq = np.asarray(Wq, dtype=np.float32)
    Wk = np.asarray(Wk, dtype=np.float32)
    Wv = np.asarray(Wv, dtype=np.float32)
    Wp = np.asarray(Wp, dtype=np.float32)
    bp = np.asarray(bp, dtype=np.float32)
    scale_mul = np.asarray(scale_mul, dtype=np.float32)

    sm = np.exp(np.minimum(scale_mul, MAX_SCALE_MUL)).reshape(H)  # (H,)
    expb = np.exp(attn_bias[0])  # (H, L, Lc)

    perm = _col_perm()
    # hsum/hbc map partitions of tile t to heads under the new order
    hsum = np.zeros((OC, HG), dtype=BF16)
    hbc = np.zeros((HG, OC), dtype=BF16)
    for t in range(4):
        for p in range(128):
            h = 4 * (t // 2) + p // 32
            hsum[t * 128 + p, h] = 1.0
            hbc[h, t * 128 + p] = 1.0

    gshard = {}
    for g in range(2):
        rows = slice(g * OC, (g + 1) * OC)
        heads = slice(g * HG, (g + 1) * HG)
        wq_g = (WSCALE * Wq[rows, :])[perm, :]  # [OC, C] reordered rows
        wk_g = (WSCALE * Wk[rows, :])[perm, :]
        gshard[g] = dict(
            wqT=np.ascontiguousarray(wq_g.T).astype(F8),
            wkT=np.ascontiguousarray(wk_g.T).astype(F8),
            wvT=np.ascontiguousarray(Wv[rows, :].T).astype(BF16),
            wpT=np.ascontiguousarray(Wp[:, rows].T).astype(BF16),
            expbT=np.ascontiguousarray(
                np.transpose(expb[heads], (0, 2, 1))
            ).astype(BF16),
            # s = (psn2 * sminv)^-0.5 should equal R*sm/sqrt(psn2)
            sminv=(1.0 / (R * R * sm[heads] ** 2)).reshape(HG, 1).astype(np.float32),
        )
    bshard = {}
    for b in range(B):
        xb = np.ascontiguousarray(x[b].T)
        cb = np.ascontiguousarray(context[b].T)
        bshard[b] = dict(
            xT8=xb.astype(F8),
            ctxT=cb.astype(BF16),
            ctxT8=cb.astype(F8),
        )

    in_maps = []
    for core in range(N_CORES):
        b, g = core // 2, core % 2
        m = dict(hsum=hsum, hbc=hbc)
        m.update(gshard[g])
        m.update(bshard[b])
        in_maps.append(m)

    nc = _get_nc()
    res = run_bass_kernel_spmd(
        nc, in_maps, core_ids=list(range(N_CORES)), trace=TRACE
    )
    LAST_RESULT = res
    outs = [r["y"].astype(np.float32) for r in res.results]
    out = np.stack(
        [outs[2 * b] + outs[2 * b + 1] + bp[None, :] for b in range(B)]
    ).astype(np.float32)
    return out


# revision 90
# speedup vs baseline: 1.0568x; 1.0062x over previous
"""Distributed Trainium2 Bass kernel for nn_CrossAttention (B=4, L=1024,
Lc=2048, C=1024, H=16).

Sharding: 8 cores = 4 batches x 2 head-groups of 8 heads. Each core
computes its batch's q/k/v projections for its 8 heads, the attention,
and a partial output projection (row-shard of Wp). Host sums the two
partial outputs per batch (bf16) and adds bp.

Precision: q/k projections and the S=q@k^T matmul run in fp8e4 with
perf_mode=DoubleRow (two contraction subtiles per instruction -> 2x PE
throughput, K=256 per instruction for the projections). Wq/Wk are
pre-scaled by 32 on the host (q/k are l2-normalized downstream so the
scale cancels); qhat is rescaled by R=2048 into fp8 range and exp()
compensates with scale=1/R. Weight columns are reordered host-side so
each 128-partition tile holds 4 heads x 32 head-dim lanes, giving the
S matmul its [32, 2, *] DoubleRow layout directly. V/AV/P-projection
stay bf16 for accuracy.

Schedule: per head, S -> exp -> bias-multiply m-tiles stream at the
scalar engine's pace (exp is the global bottleneck at ~134us busy).
AV matmuls trail roughly one head behind via a global cursor so exp
never stalls on PE; the v projection interleaves into heads 0-1; the
first half of the output projection (head pairs j=0,1) runs in-stream
into an SBUF partial (ypart) once heads 0-3 finish; heads run in order
[0,1,2,3,4,5,7,6] so the final tail avoids the cross-partition DMA.
The softmax denominator rides as a leading ones-column in V (rowsum
lands at PSUM partition 0 for partition_broadcast); values sit at
partitions 32..95 to satisfy base-partition rules.
"""

import os
import sys
from contextlib import ExitStack

sys.path.insert(0, "/opt/trn_rl_repo")

import numpy as np
import ml_dtypes

import concourse.bass as bass
from concourse import bacc
import concourse.mybir as mybir
import concourse.tile as tile
from concourse.bass_utils import run_bass_kernel_spmd

BF16 = ml_dtypes.bfloat16
F8 = ml_dtypes.float8_e4m3
AF = mybir.ActivationFunctionType
ALU = mybir.AluOpType
AX = mybir.AxisListType
PM = mybir.MatmulPerfMode

# All ACT functions used here (Copy/Exp/Ln) live in the
# natural_log_exp_and_others table set; blank the other sets so
# insert_act_table_loads emits exactly one table load.
from concourse.hw_specs import get_activation_tables as _gat_orig


def _gat_one_set(arch):
    t = _gat_orig(arch)
    return {
        k: (v if k == "natural_log_exp_and_others" else set()) for k, v in t.items()
    }


bacc.get_activation_tables = _gat_one_set

B, L, LC, C, H = 4, 1024, 2048, 1024, 16
HG = 8  # heads per core
D = 64  # head dim
OC = HG * D  # 512 output channels per core
N_CORES = 8
MAX_SCALE_MUL = float(np.log(100.0))
WSCALE = 32.0  # host pre-scale on Wq/Wk before fp8 quantization
R = 2048.0  # qhat rescale into fp8 range; exp() applies 1/R

# module-level knobs for test harness
TRACE = False
LAST_RESULT = None

_NC_CACHE = {}


def build_nc():
    f32, bf16, f8 = mybir.dt.float32, mybir.dt.bfloat16, mybir.dt.float8e4
    nc = bacc.Bacc()

    xT8 = nc.declare_dram_parameter("xT8", [C, L], f8, isOutput=False)
    ctxT = nc.declare_dram_parameter("ctxT", [C, LC], bf16, isOutput=False)
    ctxT8 = nc.declare_dram_parameter("ctxT8", [C, LC], f8, isOutput=False)
    wqT = nc.declare_dram_parameter("wqT", [C, OC], f8, isOutput=False)
    wkT = nc.declare_dram_parameter("wkT", [C, OC], f8, isOutput=False)
    wvT = nc.declare_dram_parameter("wvT", [C, OC], bf16, isOutput=False)
    wpT = nc.declare_dram_parameter("wpT", [OC, C], bf16, isOutput=False)
    expbT = nc.declare_dram_parameter("expbT", [HG, LC, L], bf16, isOutput=False)
    hsum = nc.declare_dram_parameter("hsum", [OC, HG], bf16, isOutput=False)
    hbc = nc.declare_dram_parameter("hbc", [HG, OC], bf16, isOutput=False)
    sminv = nc.declare_dram_parameter("sminv", [HG, 1], f32, isOutput=False)
    y = nc.declare_dram_parameter("y", [L, C], bf16, isOutput=True)
    dbg = {}
    if os.environ.get("KDBG", "0") == "1":
        dbg["qhatF"] = nc.declare_dram_parameter("d_qhatF", [128, 2, L], f8, isOutput=True)
        dbg["kTF"] = nc.declare_dram_parameter("d_kTF", [128, 2, LC], f8, isOutput=True)
        dbg["ssb"] = nc.declare_dram_parameter("d_ssb", [HG, L], bf16, isOutput=True)
        dbg["rsk"] = nc.declare_dram_parameter("d_rsk", [128, 4], f32, isOutput=True)
        dbg["qT0"] = nc.declare_dram_parameter("d_qT0", [128, L], bf16, isOutput=True)
        dbg["v0"] = nc.declare_dram_parameter("d_v0", [128, HG, 96], bf16, isOutput=True)
        dbg["praw"] = nc.declare_dram_parameter("d_praw", [128, L], bf16, isOutput=True)
        dbg["ptb"] = nc.declare_dram_parameter("d_ptb", [128, L], bf16, isOutput=True)
        dbg["osb"] = nc.declare_dram_parameter("d_osb", [96, L], f32, isOutput=True)
        dbg["on2"] = nc.declare_dram_parameter("d_on2", [128, L], bf16, isOutput=True)

    KT = C // 128  # 8 contraction tiles (DoubleRow: pairs -> 4 steps)
    OCT = OC // 128  # 4 output-channel tiles
    MT = LC // 128  # 16 context tiles
    LT = L // 128  # 8 query tiles

    with tile.TileContext(nc) as tc, ExitStack() as persist:
        keep = persist.enter_context(tc.tile_pool(name="keep", bufs=1))
        dma = nc.sync

        # head PAIRS stacked across the 128 partitions for the output
        # projection: contraction becomes standard K=128 matmuls
        wp_t = keep.tile([128, HG // 2, C], bf16, tag="wp")
        wp2_sb = [wp_t[:, j, :] for j in range(HG // 2)]

        # fp8 DoubleRow layouts for the S matmul:
        # group g in {0,1} holds heads 4g..4g+3; head h4 = partitions
        # 32*h4..32*h4+31, subtile i covers head-dim 32i..32i+31.
        kTF_sb = [keep.tile([128, 2, LC], f8, tag=f"kTF{g}", name=f"kTF{g}") for g in range(2)]
        qhatF_sb = [keep.tile([128, 2, L], f8, tag=f"qhatF{g}", name=f"qhatF{g}") for g in range(2)]
        # matmul operands must have base partition in {0,32,64}; head 3 of
        # each group lives at base 96, so keep partition-shifted copies
        # (group g at base 32*g of one shared tile).
        kTF96c = keep.tile([64, 2, LC], f8, tag="kTF96c", name="kTF96c")
        qhatF96c = keep.tile([64, 2, L], f8, tag="qhatF96c", name="qhatF96c")
        kTF96_sb = [kTF96c[32 * g : 32 * g + 32, :, :] for g in range(2)]
        qhatF96_sb = [qhatF96c[32 * g : 32 * g + 32, :, :] for g in range(2)]
        v_sb = [keep.tile([128, HG, 96], bf16, tag=f"v{mt}", name=f"v{mt}") for mt in range(MT)]
        on2_sb = [keep.tile([128, L], bf16, tag=f"on2_{j}", name=f"on2_{j}") for j in range(HG // 2)]

        vpool = persist.enter_context(tc.tile_pool(name="vpool", bufs=1))
        ebpool = persist.enter_context(tc.tile_pool(name="ebpool", bufs=4))
        ebt_tiles = {}
        HMT = MT // 4  # mts per ebt quarter-tile

        def load_ebt(h, half):
            t = ebpool.tile([128, HMT, L], bf16, tag="expb", name=f"ebt{h}_{half}")
            lo = half * HMT
            dma.dma_start(
                out=t,
                in_=expbT[h, lo * 128 : (lo + HMT) * 128, :].rearrange(
                    "(g p) l -> p g l", p=128
                ),
            )
            ebt_tiles[(h, half)] = t

        # ---------------- phase 1: projections + norms ----------------
        with ExitStack() as p1:
            wpool = p1.enter_context(tc.tile_pool(name="wpool", bufs=1))
            apool = p1.enter_context(tc.tile_pool(name="apool", bufs=1))
            spool = p1.enter_context(tc.tile_pool(name="spool", bufs=1))
            psA = p1.enter_context(tc.tile_pool(name="psA", bufs=3, space="PSUM"))

            # q-phase inputs first so PE can start ASAP, then k/v inputs
            wq_t = wpool.tile([128, KT, OC], f8, tag="wq")
            wqT_r = wqT.rearrange("(t p) o -> p t o", p=128)
            dma.dma_start(out=wq_t, in_=wqT_r)
            x_t = apool.tile([128, KT, L], f8, tag="x")
            xT_r = xT8.rearrange("(t p) l -> p t l", p=128)
            dma.dma_start(out=x_t, in_=xT_r)
            wk_t = wpool.tile([128, KT, OC], f8, tag="wk")
            dma.dma_start(out=wk_t, in_=wkT.rearrange("(t p) o -> p t o", p=128))
            ctx8_t = apool.tile([128, KT, LC], f8, tag="ctx8")
            dma.dma_start(out=ctx8_t, in_=ctxT8.rearrange("(t p) m -> p t m", p=128))
            hsum_t = wpool.tile([128, OCT, HG], bf16, tag="hsum")
            dma.dma_start(out=hsum_t, in_=hsum.rearrange("(t p) h -> p t h", p=128))
            hsum_sb = [hsum_t[:, ot, :] for ot in range(OCT)]
            hbc_sb = wpool.tile([HG, OC], bf16, tag="hbc")
            dma.dma_start(out=hbc_sb, in_=hbc[:, :])
            sminv_sb = wpool.tile([HG, 1], f32, tag="sminv")
            dma.dma_start(out=sminv_sb, in_=sminv[:, :])
            # head-0 exp(bias) front chunk early so the bias-multiply
            # stream doesn't stall; remainder after ctx/wv
            load_ebt(0, 0)
            load_ebt(0, 2)
            load_ebt(0, 3)
            ctx_t = vpool.tile([128, KT, LC], bf16, tag="ctx")
            dma.dma_start(out=ctx_t, in_=ctxT.rearrange("(t p) m -> p t m", p=128))
            ctx_sb = [ctx_t[:, kt, :] for kt in range(KT)]
            wv_t = vpool.tile([128, KT, OC], bf16, tag="wv")
            dma.dma_start(out=wv_t, in_=wvT.rearrange("(t p) o -> p t o", p=128))
            wv_sb = [wv_t[:, kt, :] for kt in range(KT)]
            load_ebt(0, 1)

            # q projection (fp8 DoubleRow): qT (bf16) and q^2 (bf16)
            qT_sb, q2_sb = [], []
            for ot in range(OCT):
                ps = psA.tile([128, L], f32, tag="psA")
                oc_sl = slice(ot * 128, (ot + 1) * 128)
                for kt in range(0, KT, 2):
                    for nch in range(L // 512):
                        nsl = slice(nch * 512, (nch + 1) * 512)
                        nc.tensor.matmul(
                            ps[:, nsl],
                            wq_t[:, kt : kt + 2, oc_sl],
                            x_t[:, kt : kt + 2, nsl],
                            start=(kt == 0),
                            stop=(kt == KT - 2),
                            perf_mode=PM.DoubleRow,
                        )
                t = apool.tile([128, L], bf16, tag=f"qT{ot}")
                nc.vector.tensor_copy(t, ps)
                qT_sb.append(t)
                t2 = apool.tile([128, L], bf16, tag=f"q2{ot}")
                nc.vector.tensor_mul(t2, t, t)
                q2_sb.append(t2)

            # k projection (fp8 DoubleRow, two Lc halves per oc-tile)
            # + k row norms; evacuate straight to fp8 kTF layout
            n2k_all = spool.tile([128, OCT], f32, tag="n2k_all")
            rsk_all = spool.tile([128, OCT], f32, tag="rsk_all")
            lnk_all = spool.tile([128, OCT], f32, tag="lnk_all")

            def k_proj(ot):
                g, sub = ot // 2, ot % 2
                oc_sl = slice(ot * 128, (ot + 1) * 128)
                n2kh = spool.tile([128, 2], f32, tag=f"n2kh{ot}")
                for half in range(2):
                    ps = psA.tile([128, 1024], f32, tag="psA")
                    for kt in range(0, KT, 2):
                        for nch in range(2):
                            gsl = slice(
                                half * 1024 + nch * 512, half * 1024 + (nch + 1) * 512
                            )
                            nsl = slice(nch * 512, (nch + 1) * 512)
                            nc.tensor.matmul(
                                ps[:, nsl],
                                wk_t[:, kt : kt + 2, oc_sl],
                                ctx8_t[:, kt : kt + 2, gsl],
                                start=(kt == 0),
                                stop=(kt == KT - 2),
                                perf_mode=PM.DoubleRow,
                            )
                    kt_half = kTF_sb[g][:, sub, half * 1024 : (half + 1) * 1024]
                    nc.scalar.activation(kt_half, ps, AF.Copy)
                    k2s = spool.tile([128, 1024], f8, tag="k2s", bufs=2, name="k2s")
                    # k2s = kt*kt with fused row-sum accumulation (from the
                    # fp8 copy; quantization error on the norm is ~0.1%)
                    nc.vector.scalar_tensor_tensor(
                        k2s,
                        kt_half,
                        1.0,
                        kt_half,
                        op0=ALU.mult,
                        op1=ALU.mult,
                        accum_out=n2kh[:, half : half + 1],
                    )
                nc.vector.tensor_add(
                    n2k_all[:, ot : ot + 1], n2kh[:, 0:1], n2kh[:, 1:2]
                )

            # q norms: n2[h,l] -> s = R*sm/sqrt(n2) -> broadcast to rows
            with tc.tile_pool(name="psN", bufs=1, space="PSUM") as psN:
                psn2 = psN.tile([HG, L], f32, tag="psn2")
                for ot in range(OCT):
                    for nch in range(L // 512):
                        nsl = slice(nch * 512, (nch + 1) * 512)
                        nc.tensor.matmul(
                            psn2[:, nsl],
                            hsum_sb[ot],
                            q2_sb[ot][:, nsl],
                            start=(ot == 0),
                            stop=(ot == OCT - 1),
                        )
                k_proj(0)
                k_proj(1)
                k_proj(2)
                k_proj(3)
                nc.scalar.activation(lnk_all, n2k_all, AF.Ln)
                nc.scalar.activation(rsk_all, lnk_all, AF.Exp, scale=-0.5)
                t8 = spool.tile([HG, L], bf16, tag="t8")
                nc.scalar.activation(t8, psn2, AF.Ln, scale=sminv_sb[:, 0:1])
            s_sb = spool.tile([HG, L], bf16, tag="s_sb")
            nc.scalar.activation(s_sb, t8, AF.Exp, scale=-0.5)
            sbc_sb = []
            for ot in range(OCT):
                ps = psA.tile([128, L], f32, tag="psA")
                for nch in range(L // 512):
                    nsl = slice(nch * 512, (nch + 1) * 512)
                    nc.tensor.matmul(
                        ps[:, nsl],
                        hbc_sb[:, ot * 128 : (ot + 1) * 128],
                        s_sb[:, nsl],
                        start=True,
                        stop=True,
                    )
                sbc = spool.tile([128, L], bf16, tag=f"sbc{ot}", name="sbc")
                nc.vector.tensor_copy(sbc, ps)
                sbc_sb.append(sbc)

            # qhat = (qT * rsk_per_partition) * s_broadcast -> fp8 layout
            for ot in range(OCT):
                g, sub = ot // 2, ot % 2
                nc.vector.scalar_tensor_tensor(
                    qhatF_sb[g][:, sub, :],
                    qT_sb[ot],
                    rsk_all[:, ot : ot + 1],
                    sbc_sb[ot],
                    op0=ALU.mult,
                    op1=ALU.mult,
                )
            # partition-shifted copies of head 3 (base 96 -> 0)
            for g in range(2):
                dma.dma_start(out=kTF96_sb[g], in_=kTF_sb[g][96:128, :, :])
                dma.dma_start(out=qhatF96_sb[g], in_=qhatF_sb[g][96:128, :, :])
            if dbg:
                dma.dma_start(out=dbg["qhatF"][:, :, :], in_=qhatF_sb[0])
                dma.dma_start(out=dbg["kTF"][:, :, :], in_=kTF_sb[0])
                dma.dma_start(out=dbg["ssb"][:, :], in_=s_sb)
                dma.dma_start(out=dbg["rsk"][:, :], in_=rsk_all)
                dma.dma_start(out=dbg["qT0"][:, :], in_=qT_sb[0])


        # ---------------- phase 2: attention ----------------
        # Per head: S -> exp -> bias-mult m-tiles stream at the scalar
        # engine's pace (one m-tile per PSUM tile). AV matmuls trail
        # roughly one head behind via a global cursor so exp never waits
        # on PE; the v projection slots in after head 0's stream.
        with ExitStack() as p2:
            stpool = p2.enter_context(tc.tile_pool(name="stream", bufs=4))
            ypool2 = p2.enter_context(tc.tile_pool(name="ypool2", bufs=1))
            yp_box = {"ypart": ypool2.tile([128, LT, C], bf16, tag="ypart", name="ypart")}
            ptbpool = p2.enter_context(tc.tile_pool(name="ptbpool", bufs=17))
            tpool = p2.enter_context(tc.tile_pool(name="tails", bufs=1))
            psS = p2.enter_context(tc.tile_pool(name="psS", bufs=2, space="PSUM"))
            psO = p2.enter_context(tc.tile_pool(name="psO", bufs=1, space="PSUM"))

            ptb_all = {}  # (h, mt) -> ptb tile
            pso_all = {}  # h -> AV psum accumulator
            av_queue = []  # (h, mt), strictly ordered
            emitted_mts = [0] * HG
            v_state = {"done": 0, "psV": None}

            def v_proj_mt(mt):
                # v projection for one m-tile into (m, head, 1+d) layout
                # (ones column first so the AV rowsum lands at partition 0)
                ps = v_state["psV"].tile([128, OC], f32, tag="psV", name="psV")
                msl = slice(mt * 128, (mt + 1) * 128)
                for kt in range(KT):
                    nc.tensor.matmul(
                        ps,
                        ctx_sb[kt][:, msl],
                        wv_sb[kt],
                        start=(kt == 0),
                        stop=(kt == KT - 1),
                    )
                nc.scalar.activation(
                    v_sb[mt][:, :, 32:96],
                    ps.rearrange("p (h d) -> p h d", h=HG),
                    AF.Copy,
                )
                # col 0 = ones (rowsum lands at pso partition 0); cols
                # 1-31 are dead padding so values start at base 32.
                nc.vector.memset(v_sb[mt][:, :, 0:1], 1.0)
                nc.vector.memset(v_sb[mt][:, :, 1:32], 0.0)
                if dbg and mt == 0:
                    dma.dma_start(out=dbg["v0"][:, :, :], in_=v_sb[0])
                v_state["done"] = mt + 1

            tails_done = [0]

            def emit_tail(h):
                tails_done[0] += 1
                pso = pso_all.pop(h)
                # evacuate pso right away so its PSUM banks free for the
                # next head; tail math runs from SBUF. The last head (6)
                # runs the chain in L-halves on DVE so the output
                # projection's first row-blocks can start sooner.
                last = h == 6
                osb = tpool.tile([96, L], f32, tag="osb", bufs=2, name="osb")
                rrec = tpool.tile([1, L], bf16, tag="rrec", bufs=1, name="rrec")
                rb = tpool.tile([96, L], bf16, tag="rb", bufs=1, name="rb")
                if h % 2 == 0:
                    dst = on2_sb[h // 2][0:D, :]
                else:
                    dst = tpool.tile([D, L], bf16, tag="onodd", bufs=1, name="onodd")
                eng = nc.vector if last else nc.gpsimd
                for sl in ([slice(0, 256), slice(256, 512), slice(512, 768), slice(768, L)] if last else [slice(0, L)]):
                    # last head: read straight from PSUM (skip the copy);
                    # earlier heads evacuate so the pso banks recycle.
                    src = pso if last else osb
                    if not last:
                        nc.vector.tensor_copy(osb[:, sl], pso[:, sl])
                    # rowsum sits at partition 0 (ones column is v[..., 0])
                    with nc.allow_low_precision(reason="bf16 rowsum recip"):
                        nc.vector.reciprocal(rrec[:, sl], src[0:1, sl])
                    nc.gpsimd.partition_broadcast(rb[:, sl], rrec[:, sl], channels=96)
                    # base-32 APs may touch at most 32 partitions: halves
                    eng.tensor_mul(dst[0:32, sl], src[32:64, sl], rb[32:64, sl])
                    eng.tensor_mul(dst[32:64, sl], src[64:96, sl], rb[64:96, sl])
                if h % 2 == 1:
                    dma.dma_start(out=on2_sb[h // 2][D:128, :], in_=dst)
                if dbg and h == 0:
                    dma.dma_start(out=dbg["osb"][:, :], in_=osb)
                if dbg and h == 1:
                    dma.dma_start(out=dbg["on2"][:, :], in_=on2_sb[0])

            def emit_avs(budget):
                while budget > 0 and av_queue:
                    h, mt = av_queue[0]
                    if emitted_mts[h] <= mt or v_state["done"] <= mt:
                        return
                    if h not in pso_all:
                        pso_all[h] = psO.tile([96, L], f32, tag="pso", name="pso")
                    pso = pso_all[h]
                    ptb_tile = ptb_all.pop((h, mt))
                    for nch in range(2):
                        nsl = slice(nch * 512, (nch + 1) * 512)
                        nc.tensor.matmul(
                            pso[:, nsl],
                            v_sb[mt][:, h, :],
                            ptb_tile[:, nsl],
                            start=(mt == 0),
                            stop=(mt == MT - 1),
                        )
                    av_queue.pop(0)
                    budget -= 1
                    if mt == MT - 1:
                        emit_tail(h)

            psYa_box = {"pool": None, "lt": 0}
            v_state["psVcm"] = tc.tile_pool(name="psV", bufs=2, space="PSUM")
            v_state["psV"] = v_state["psVcm"].__enter__()

            def emit_ppass1(lt):
                # first half of the output projection (heads 0..3) into
                # ypart while later heads stream; phase 3 adds j=2,3.
                lsl = slice(lt * 128, (lt + 1) * 128)
                psy = psYa_box["pool"].tile(
                    [128, C], f32, tag="psya", name="psya", bufs=1
                )
                for j in range(2):
                    for nch in range(C // 512):
                        nsl = slice(nch * 512, (nch + 1) * 512)
                        nc.tensor.matmul(
                            psy[:, nsl],
                            on2_sb[j][:, lsl],
                            wp2_sb[j][:, nsl],
                            start=(j == 0),
                            stop=(j == 1),
                        )
                nc.vector.tensor_copy(yp_box["ypart"][:, lt, :], psy)

            HEAD_ORDER = [0, 1, 2, 3, 4, 5, 7, 6]
            for hidx, hh in enumerate(HEAD_ORDER):
                nxt = HEAD_ORDER[hidx + 1] if hidx + 1 < HG else None
                g, h4 = hh // 4, hh % 4
                psl = slice(32 * h4, 32 * h4 + 32)
                if h4 == 3:
                    k_src, q_src = kTF96_sb[g], qhatF96_sb[g]
                else:
                    k_src = kTF_sb[g][psl, :, :]
                    q_src = qhatF_sb[g][psl, :, :]
                if hh == 4:
                    psYa_box["pool"] = p2.enter_context(
                        tc.tile_pool(name="psYa", bufs=1, space="PSUM")
                    )

                if hh == 1:
                    # wp is only read by the output projection
                    dma.dma_start(
                        out=wp_t, in_=wpT.rearrange("(j p) o -> p j o", p=128)
                    )
                for mt in range(MT):
                    # prefetch next head's quarter q-1 only after this
                    # head's quarter q-1 is fully consumed (buffer reuse)
                    if mt % HMT == 0 and mt > 0 and nxt is not None:
                        load_ebt(nxt, mt // HMT - 1)
                    msl = slice(mt * 128, (mt + 1) * 128)
                    pss = psS.tile([128, L], f32, tag="pss", name="pss")
                    with tc.high_priority(offset=100):
                        for nch in range(2):
                            nsl = slice(nch * 512, (nch + 1) * 512)
                            nc.tensor.matmul(
                                pss[:, nsl],
                                k_src[:, :, msl],
                                q_src[:, :, nsl],
                                start=True,
                                stop=True,
                                perf_mode=PM.DoubleRow,
                            )
                    praw = stpool.tile([128, L], bf16, tag="praw", name="praw")
                    nc.scalar.activation(praw, pss, AF.Exp, scale=1.0 / R)
                    n_live = len(ptb_all) + 1
                    assert n_live <= 17, f"ptb live {n_live} exceeds pool bufs"
                    ptb = ptbpool.tile([128, L], bf16, tag="ptb", name="ptb")
                    nc.vector.tensor_mul(
                        ptb, praw, ebt_tiles[(hh, mt // HMT)][:, mt % HMT, :]
                    )
                    if mt % HMT == HMT - 1:
                        ebt_tiles.pop((hh, mt // HMT))
                    if dbg and hh == 0 and mt == 0:
                        dma.dma_start(out=dbg["praw"][:, :], in_=praw)
                        dma.dma_start(out=dbg["ptb"][:, :], in_=ptb)
                    ptb_all[(hh, mt)] = ptb
                    emitted_mts[hh] = mt + 1
                    av_queue.append((hh, mt))
                    if hh == 0 and mt >= 6:
                        v_proj_mt(mt - 6)
                    if hh == 1 and 10 + mt // 2 < MT and mt % 2 == 0:
                        v_proj_mt(10 + mt // 2)
                    if (
                        psYa_box["lt"] < LT
                        and psYa_box["pool"] is not None
                        and tails_done[0] >= 4
                        and (mt % 8 == 6)
                    ):
                        emit_ppass1(psYa_box["lt"])
                        psYa_box["lt"] += 1
                    emit_avs(3 if len(av_queue) > 12 or hidx == HG - 1 else (2 if len(av_queue) > 6 else 1))

                if nxt is not None:
                    load_ebt(nxt, 3)
                if hh == 1:
                    v_state["psVcm"].__exit__(None, None, None)

            while av_queue:
                emit_avs(1000)
            while psYa_box["lt"] < LT:
                emit_ppass1(psYa_box["lt"])
                psYa_box["lt"] += 1

            # ---------------- phase 3: output projection ----------------
            ypool = p2.enter_context(tc.tile_pool(name="ypool", bufs=2))

            for lt in range(LT):
                lsl = slice(lt * 128, (lt + 1) * 128)
                ysb = ypool.tile([128, C], bf16, tag="ysb")
                psy = psS.tile([128, L], f32, tag="pss", name="pss")
                for j in range(2, HG // 2):
                    for nch in range(C // 512):
                        nsl = slice(nch * 512, (nch + 1) * 512)
                        nc.tensor.matmul(
                            psy[:, nsl],
                            on2_sb[j][:, lsl],
                            wp2_sb[j][:, nsl],
                            start=(j == 2),
                            stop=(j == HG // 2 - 1),
                        )
                nc.vector.tensor_add(ysb, psy, yp_box["ypart"][:, lt, :])
                dma.dma_start(out=y[lsl, :], in_=ysb)

    nc.compile()
    return nc


def _get_nc():
    if "nc" not in _NC_CACHE:
        _NC_CACHE["nc"] = build_nc()
    return _NC_CACHE["nc"]


def _col_perm():
    """New column order within a core's OC block: tile t in 0..3 holds
    heads 4*(t//2)..4*(t//2)+3, head-dim lanes 32*(t%2)..32*(t%2)+31;
    partition p maps to (h = 4*(t//2) + p//32, d = 32*(t%2) + p%32).
    Returns perm with perm[t*128 + p] = old column index h*64 + d."""
    perm = np.empty(OC, dtype=np.int64)
    for t in range(4):
        for p in range(128):
            h = 4 * (t // 2) + p // 32
            d = 32 * (t % 2) + p % 32
            perm[t * 128 + p] = h * 64 + d
    return perm


def kernel(x, context, attn_bias, Wq, Wk, Wv, Wp, bp, scale_mul):
    global LAST_RESULT
    x = np.asarray(x, dtype=np.float32)
    context = np.asarray(context, dtype=np.float32)
    attn_bias = np.asarray(attn_bias, dtype=np.float32)
    Wq = np.asarray(Wq, dtype=np.float32)
    Wk = np.asarray(Wk, dtype=np.float32)
    Wv = np.asarray(Wv, dtype=np.float32)
    Wp = np.asarray(Wp, dtype=np.float32)
    bp = np.asarray(bp, dtype=np.float32)
    scale_mul = np.asarray(scale_mul, dtype=np.float32)

    sm = np.exp(np.minimum(scale_mul, MAX_SCALE_MUL)).reshape(H)  # (H,)
    expb = np.exp(attn_bias[0])  # (H, L, Lc)

    perm = _col_perm()
    # hsum/hbc map partitions of tile t to heads under the new order
    hsum = np.zeros((OC, HG), dtype=BF16)
    hbc = np.zeros((HG, OC), dtype=BF16)
    for t in range(4):
        for p in range(128):
            h = 4 * (t // 2) + p // 32
            hsum[t * 128 + p, h] = 1.0
            hbc[h, t * 128 + p] = 1.0

    gshard = {}
    for g in range(2):
        rows = slice(g * OC, (g + 1) * OC)
        heads = slice(g * HG, (g + 1) * HG)
        wq_g = (WSCALE * Wq[rows, :])[perm, :]  # [OC, C] reordered rows
        wk_g = (WSCALE * Wk[rows, :])[perm, :]
        gshard[g] = dict(
            wqT=np.ascontiguousarray(wq_g.T).astype(F8),
            wkT=np.ascontiguousarray(wk_g.T).astype(F8),
            wvT=np.ascontiguousarray(Wv[rows, :].T).astype(BF16),
            wpT=np.ascontiguousarray(Wp[:, rows].T).astype(BF16),
            expbT=np.ascontiguousarray(
                np.transpose(expb[heads], (0, 2, 1))
            ).astype(BF16),
            # s = (psn2 * sminv)^-0.5 should equal R*sm/sqrt(psn2)
            sminv=(1.0 / (R * R * sm[heads] ** 2)).reshape(HG, 1).astype(np.float32),
        )
    bshard = {}
    for b in range(B):
        xb = np.ascontiguousarray(x[b].T)
        cb = np.ascontiguousarray(context[b].T)
        bshard[b] = dict(
            xT8=xb.astype(F8),
            ctxT=cb.astype(BF16),
            ctxT8=cb.astype(F8),
        )

    in_maps = []
    for core in range(N_CORES):
        b, g = core // 2, core % 2
        m = dict(hsum=hsum, hbc=hbc)
        m.update(gshard[g])
        m.update(bshard[b])
        in_maps.append(m)

    nc = _get_nc()
    res = run_bass_kernel_spmd(
        nc, in_maps, core_ids=list(range(N_CORES)), trace=TRACE
    )
    LAST_RESULT = res
    outs = [r["y"].astype(np.float32) for r in res.results]
    out = np.stack(
        [outs[2 * b] + outs[2 * b + 1] + bp[None, :] for b in range(B)]
    ).astype(np.float32)
    return out
